# revision 13
# baseline (speedup 1.0000x reference)
"""AttnDecoderRNN Trainium2 kernel.

B=32, S=64, T=64, H=512, V=32000. 8 NeuronCores, batch-sharded (4 per core).

Per core:
  Phase 0: precompute Ua@keys (transposed), gie = E@W_ihA.T + bias,
           KW = keys@W_ihC.T (pair-stacked), load combined [Wa|W_hh].T.
  Phase 1: 64 sequential GRU+attention steps. Gate matmuls keep h as the
           (tiny) stationary operand and stream the weights; the four gate
           regions go to four PSUM partition bands via tile_position
           col-groups so they run concurrently on the PE sub-arrays.
  Phase 2: logits = hT @ out_w.T in fp16 (batched over 32 timesteps per
           m-tile), log_softmax with a constant 25.0 shift (exact: shift
           cancels), fused exp+accumulate on ACT.
"""

import math
import os
import sys
from contextlib import ExitStack

import numpy as np

sys.path.insert(0, "/opt/trn_rl_repo")

import concourse.bass as bass
import concourse.bacc as bacc
import concourse.mybir as mybir
import concourse.tile as tile
from concourse.bass_utils import run_bass_kernel_spmd
from concourse.masks import make_identity

F32 = mybir.dt.float32
F16 = mybir.dt.float16
AF = mybir.ActivationFunctionType
ALU = mybir.AluOpType
AX = mybir.AxisListType

B, S, T, H, V = 32, 64, 64, 512, 32000
NCORES = 8
BL = B // NCORES          # 4 batches per core
KT = H // 128             # 4 k-tiles
NV = 512                  # v-chunk
NVC = V // NV             # 62.5 -> handle tail
VCHUNKS = [(i * NV, min(NV, V - i * NV)) for i in range((V + NV - 1) // NV)]
LOGIT_SHIFT = 25.0

USE_COLTILE = True


def _build_program():
    nc = bacc.Bacc("TRN2", target_bir_lowering=False, debug=False,
                   enable_asserts=False, num_devices=NCORES)

    # ---- DRAM I/O ----
    d_keys = nc.dram_tensor("keys", [BL, S, H], F32, kind="ExternalInput")
    d_h0 = nc.dram_tensor("h0", [BL, H], F32, kind="ExternalInput")
    d_eg = nc.dram_tensor("eg", [T * BL, H], F32, kind="ExternalInput")
    d_wmov = nc.dram_tensor("wmovT", [H, 4 * H], F32, kind="ExternalInput")
    d_wihaT = nc.dram_tensor("wihaT", [H, 3 * H], F32, kind="ExternalInput")
    d_wihcT = nc.dram_tensor("wihcT", [H, 3 * H], F32, kind="ExternalInput")
    d_uaT = nc.dram_tensor("uaT", [H, H], F32, kind="ExternalInput")
    d_bihh = nc.dram_tensor("bihh", [3, H], F32, kind="ExternalInput")
    d_bhhn = nc.dram_tensor("bhhn", [1, H], F32, kind="ExternalInput")
    d_uabT = nc.dram_tensor("uabT", [128, KT], F32, kind="ExternalInput")
    d_vaT = nc.dram_tensor("vaT", [128, KT], F32, kind="ExternalInput")
    d_outwT = nc.dram_tensor("outwT", [H, V], F16, kind="ExternalInput")
    d_ob = nc.dram_tensor("ob", [63, 512], F16, kind="ExternalInput")

    d_out = nc.dram_tensor("out", [T * BL, V], F32, kind="ExternalOutput")
    d_attn = nc.dram_tensor("attn", [BL, T, S], F32, kind="ExternalOutput")
    d_hT = nc.dram_tensor("hout", [BL, H], F32, kind="ExternalOutput")

    with tile.TileContext(nc) as tc, ExitStack() as ctx:
        res = ctx.enter_context(tc.tile_pool(name="res", bufs=1))

        # ---------- resident tiles ----------
        ident = res.tile([128, 128], F32)
        make_identity(nc, ident)
        wmov = res.tile([128, KT, 4 * H], F32)       # [Wa|Whh].T  (q r z n)
        for kt in range(KT):
            nc.sync.dma_start(out=wmov[:, kt, :], in_=d_wmov[kt * 128:(kt + 1) * 128, :])
        uabT = res.tile([128, KT], F32)
        nc.sync.dma_start(out=uabT, in_=d_uabT.ap())
        vaT = res.tile([128, KT], F32)
        nc.sync.dma_start(out=vaT, in_=d_vaT.ap())
        bihh = []
        for i in range(3):
            bihh_i = res.tile([1, H], F32, tag=f"bihh{i}", name=f"bihh{i}")
            nc.sync.dma_start(out=bihh_i, in_=d_bihh[i:i + 1, :])
            bihh.append(bihh_i)
        bhhn = res.tile([1, H], F32)
        nc.sync.dma_start(out=bhhn, in_=d_bhhn.ap())
        ones1 = res.tile([1, 128], F32)
        nc.vector.memset(ones1, 1.0)
        ones16 = res.tile([1, 128], F16)
        nc.vector.memset(ones16, 1.0)
        neg25 = res.tile([128, 1], F32)
        nc.vector.memset(neg25, -LOGIT_SHIFT)

        keysT = res.tile([128, KT, BL * S], F32)     # [h, (b-major, s)]
        uakT = res.tile([128, KT, BL * S], F32)      # Ua@keys + (Ua_b + Wa_b)
        gie = res.tile([128, 2, 3 * H], F32)         # E@W_ihA.T + bihh, rows (t%32)*4+b
        kwT = res.tile([128, 2, 3 * H], F32)         # pair-stacked keys@W_ihC.T
        hT_hist = res.tile([128, KT, T + 1, BL], F32)
        h_nar = res.tile([BL, 2, H], F32)
        wT4 = res.tile([128, 2, 4], F32)
        nc.vector.memset(wT4, 0.0)

        # ---------- phase 0 ----------
        with tc.tile_pool(name="p0sb", bufs=3) as p0sb, \
             tc.tile_pool(name="p0ps", bufs=1, space="PSUM") as p0ps, \
             tc.tile_pool(name="p0w", bufs=3) as p0w:

            # keys -> sbuf natural [s, b, h]
            keys_nat = p0sb.tile([S, BL, H], F32, tag="knat")
            nc.sync.dma_start(out=keys_nat, in_=d_keys.ap().rearrange("b s h -> s b h"))
            # h0
            nc.sync.dma_start(out=h_nar[:, 0, :], in_=d_h0.ap())
            # E gathered [m, h], m = t*BL + b ; two m-tiles
            e_nat = p0sb.tile([128, 2, H], F32, tag="enat")
            for mt in range(2):
                nc.sync.dma_start(out=e_nat[:, mt, :], in_=d_eg[mt * 128:(mt + 1) * 128, :])

            # transposes: keysT
            for b in range(BL):
                for kt in range(KT):
                    ps = p0ps.tile([128, 128], F32, tag="tp")
                    nc.tensor.transpose(ps[:, 0:S], keys_nat[:, b, kt * 128:(kt + 1) * 128],
                                        ident[0:S, 0:S])
                    nc.vector.tensor_copy(keysT[:, kt, b * S:(b + 1) * S], ps[:, 0:S])
            # transposes: E_T [k, m]
            eT = p0sb.tile([128, KT, 2 * 128], F32, tag="eT")
            for mt in range(2):
                for kt in range(KT):
                    ps = p0ps.tile([128, 128], F32, tag="tp")
                    nc.tensor.transpose(ps, e_nat[:, mt, kt * 128:(kt + 1) * 128],
                                        ident)
                    nc.vector.tensor_copy(eT[:, kt, mt * 128:(mt + 1) * 128], ps)
            # h0 transposed into hist slot 0
            for kt in range(KT):
                ps = p0ps.tile([128, 128], F32, tag="tp")
                nc.tensor.transpose(ps[:, 0:BL], h_nar[:, 0, kt * 128:(kt + 1) * 128],
                                    ident[0:BL, 0:BL])
                nc.vector.tensor_copy(hT_hist[:, kt, 0, :], ps[:, 0:BL])

            # uakT = Ua @ keys.T + (Ua_b + Wa_b) per-partition
            for it in range(KT):
                ps = p0ps.tile([128, BL * S], F32, tag="uak")
                for kt in range(KT):
                    w = p0w.tile([128, 128], F32, tag="uaw")
                    nc.sync.dma_start(out=w, in_=d_uaT[kt * 128:(kt + 1) * 128,
                                                       it * 128:(it + 1) * 128])
                    nc.tensor.matmul(ps, w, keysT[:, kt, :],
                                     start=(kt == 0), stop=(kt == KT - 1))
                nc.vector.tensor_scalar_add(uakT[:, it, :], ps, uabT[:, it:it + 1])

            # gie = E @ W_ihA.T + bihh   [m, 3H]
            for mt in range(2):
                for ch in range(3):
                    ps = p0ps.tile([128, 512], F32, tag="gie")
                    for kt in range(KT):
                        w = p0w.tile([128, 512], F32, tag="wstream")
                        nc.sync.dma_start(out=w, in_=d_wihaT[kt * 128:(kt + 1) * 128,
                                                             ch * 512:(ch + 1) * 512])
                        nc.tensor.matmul(ps, eT[:, kt, mt * 128:(mt + 1) * 128], w,
                                         start=(kt == 0), stop=False)
                    nc.tensor.matmul(ps, ones1, bihh[ch],
                                     start=False, stop=True)
                    nc.vector.tensor_copy(gie[:, mt, ch * 512:(ch + 1) * 512], ps)

            # kwT = keys @ W_ihC.T  pair-stacked [(2b,s), 3H]
            for pr in range(2):
                for ch in range(3):
                    ps = p0ps.tile([128, 512], F32, tag="kw")
                    for kt in range(KT):
                        w = p0w.tile([128, 512], F32, tag="wstream")
                        nc.sync.dma_start(out=w, in_=d_wihcT[kt * 128:(kt + 1) * 128,
                                                             ch * 512:(ch + 1) * 512])
                        nc.tensor.matmul(ps, keysT[:, kt, pr * 128:(pr + 1) * 128], w,
                                         start=(kt == 0), stop=(kt == KT - 1))
                    nc.vector.tensor_copy(kwT[:, pr, ch * 512:(ch + 1) * 512], ps)

        # ---------- phases 1+2 ----------
        # band layout in pg psum tile [128, 1024]:
        #  rows 0:4   free 0:512 q     | free 512:1024 nacc (KW_n + gie_n)
        #  rows 32:36 free 0:512 r     (Wmov_r + KW_r + gie_r)
        #  rows 64:68 free 0:512 z
        #  rows 96:100 free 0:512 ghn  (Wmov_n + b_hh_n)
        BANDQ, BANDR, BANDZ, BANDN = 0, 32, 64, 96

        def tp(g):
            return (0, g) if USE_COLTILE else (0, 0)

        pg_pool = ctx.enter_context(tc.tile_pool(name="pg", bufs=1, space="PSUM"))
        pt_pool = ctx.enter_context(tc.tile_pool(name="pt", bufs=3, space="PSUM"))
        sc_pool = ctx.enter_context(tc.tile_pool(name="sc", bufs=1, space="PSUM"))
        pl_pool = ctx.enter_context(tc.tile_pool(name="pl", bufs=2, space="PSUM"))
        sb = ctx.enter_context(tc.tile_pool(name="stepsb", bufs=1))
        sb2 = ctx.enter_context(tc.tile_pool(name="stepsb2", bufs=2))
        l16_pool = ctx.enter_context(tc.tile_pool(name="l16", bufs=1))
        ph2sb = ctx.enter_context(tc.tile_pool(name="ph2sb", bufs=6))
        scrsb = ctx.enter_context(tc.tile_pool(name="scrsb", bufs=2))
        outsb = ctx.enter_context(tc.tile_pool(name="outsb", bufs=2))

        logits16 = l16_pool.tile([128, V], F16)
        zp = l16_pool.tile([128, 2, len(VCHUNKS)], F32)
        hT16 = l16_pool.tile([128, KT, 128], F16)

        def emit_step(t):
            pg = pg_pool.tile([128, 1024], F32, tag="pg")
            hT_prev = hT_hist[:, :, t, :]
            # q band
            for kt in range(KT):
                nc.tensor.matmul(pg[BANDQ:BANDQ + BL, 0:512], hT_prev[:, kt, :],
                                 wmov[:, kt, 0:512],
                                 start=(kt == 0), stop=(kt == KT - 1),
                                 tile_position=tp(0))
            # r band (Wmov part; KW/gie later)
            for kt in range(KT):
                nc.tensor.matmul(pg[BANDR:BANDR + BL, 0:512], hT_prev[:, kt, :],
                                 wmov[:, kt, 512:1024],
                                 start=(kt == 0), stop=False, tile_position=tp(32))
            # z band
            for kt in range(KT):
                nc.tensor.matmul(pg[BANDZ:BANDZ + BL, 0:512], hT_prev[:, kt, :],
                                 wmov[:, kt, 1024:1536],
                                 start=(kt == 0), stop=False, tile_position=tp(64))
            # ghn band + b_hh_n
            for kt in range(KT):
                nc.tensor.matmul(pg[BANDN:BANDN + BL, 0:512], hT_prev[:, kt, :],
                                 wmov[:, kt, 1536:2048],
                                 start=(kt == 0), stop=False, tile_position=tp(96))
            nc.tensor.matmul(pg[BANDN:BANDN + BL, 0:512], ones1[:, 0:BL], bhhn,
                             start=False, stop=True, tile_position=tp(96))

            # ---- attention ----
            q_sb = sb.tile([BL, H], F32, tag="qsb")
            nc.scalar.copy(q_sb, pg[BANDQ:BANDQ + BL, 0:512])
            s_tanh = sb2.tile([128, KT, BL, S], F32, tag="stanh")
            qT = sb2.tile([128, KT, BL], F32, tag="qT")
            for kt in range(KT):
                qps = pt_pool.tile([128, BL], F32, tag="tps")
                nc.tensor.transpose(qps, q_sb[:, kt * 128:(kt + 1) * 128],
                                    ident[0:BL, 0:BL])
                nc.vector.tensor_copy(qT[:, kt, :], qps)
                qb = bass.AP(tensor=qT.tensor, offset=qT[:, kt, :].offset,
                             ap=[qT.ap[0], [1, BL], [0, S]])
                nc.vector.tensor_tensor(
                    out=s_tanh[:, kt, :, :],
                    in0=uakT[:, kt, :].rearrange("p (b s) -> p b s", b=BL),
                    in1=qb, op=ALU.add)
                nc.scalar.activation(s_tanh[:, kt, :, :], s_tanh[:, kt, :, :], AF.Tanh)
            ps_s = sc_pool.tile([1, BL * S], F32, tag="scores")
            for kt in range(KT):
                nc.tensor.matmul(ps_s, vaT[:, kt:kt + 1],
                                 s_tanh[:, kt, :, :].rearrange("p b s -> p (b s)"),
                                 start=(kt == 0), stop=(kt == KT - 1),
                                 tile_position=tp(0))
            exps = sb2.tile([1, BL, S], F32, tag="exps")
            nc.scalar.activation(exps.rearrange("p b s -> p (b s)"), ps_s, AF.Exp)
            zr = sb2.tile([1, BL], F32, tag="zr")
            nc.vector.tensor_reduce(zr, exps, axis=AX.X, op=ALU.add)
            zrec = sb2.tile([1, BL], F32, tag="zrec")
            nc.vector.reciprocal(zrec, zr)
            wn = sb2.tile([1, BL, S], F32, tag="wn")
            zb = bass.AP(tensor=zrec.tensor, offset=zrec.offset,
                         ap=[zrec.ap[0], [1, BL], [0, S]])
            nc.vector.tensor_tensor(out=wn, in0=exps, in1=zb, op=ALU.mult)
            nc.sync.dma_start(out=d_attn[:, t, :], in_=wn)
            # w transposed into the zero-padded stationary tiles
            for pr in range(2):
                wps = pt_pool.tile([128, BL], F32, tag="tps")
                nc.tensor.transpose(wps[:, 0:1],
                                    wn.rearrange("p b s -> p (b s)")[:, pr * 128:(pr + 1) * 128],
                                    ident[0:1, 0:1])
                nc.vector.tensor_copy(wT4[0:S, pr, 2 * pr:2 * pr + 1], wps[0:S, 0:1])
                nc.vector.tensor_copy(wT4[S:128, pr, 2 * pr + 1:2 * pr + 2], wps[S:128, 0:1])

            # ---- KW + gie accumulation into bands ----
            t32 = t % 32
            mt = t // 32
            gie_st = sb2.tile([BL, 3 * H], F32, tag="giest")
            nc.sync.dma_start(out=gie_st, in_=gie[4 * t32:4 * t32 + BL, mt, :])
            for ch, band, grp in ((0, BANDR, 32), (1, BANDZ, 64)):
                for pr in range(2):
                    nc.tensor.matmul(pg[band:band + BL, 0:512], wT4[:, pr, :],
                                     kwT[:, pr, ch * 512:(ch + 1) * 512],
                                     start=False, stop=False, tile_position=tp(grp))
                nc.tensor.matmul(pg[band:band + BL, 0:512], ident[0:BL, 0:BL],
                                 gie_st[:, ch * 512:(ch + 1) * 512],
                                 start=False, stop=True, tile_position=tp(grp))
            # nacc band (free 512:1024)
            for pr in range(2):
                nc.tensor.matmul(pg[BANDQ:BANDQ + BL, 512:1024], wT4[:, pr, :],
                                 kwT[:, pr, 1024:1536],
                                 start=(pr == 0), stop=False, tile_position=tp(0))
            nc.tensor.matmul(pg[BANDQ:BANDQ + BL, 512:1024], ident[0:BL, 0:BL],
                             gie_st[:, 1024:1536],
                             start=False, stop=True, tile_position=tp(0))

            # ---- gates ----
            r_s = sb.tile([BL, H], F32, tag="rs")
            nc.scalar.activation(r_s, pg[BANDR:BANDR + BL, 0:512], AF.Sigmoid)
            z_s = sb.tile([BL, H], F32, tag="zs")
            nc.scalar.activation(z_s, pg[BANDZ:BANDZ + BL, 0:512], AF.Sigmoid)
            nh = sb.tile([BL, H], F32, tag="nh")
            nc.vector.tensor_tensor(out=nh, in0=r_s, in1=pg[BANDN:BANDN + BL, 0:512],
                                    op=ALU.mult)
            npre = sb.tile([BL, H], F32, tag="npre")
            nc.vector.tensor_tensor(out=npre, in0=nh, in1=pg[BANDQ:BANDQ + BL, 512:1024],
                                    op=ALU.add)
            n_s = sb.tile([BL, H], F32, tag="ns")
            nc.scalar.activation(n_s, npre, AF.Tanh)
            d_t = sb.tile([BL, H], F32, tag="dt")
            nc.vector.tensor_tensor(out=d_t, in0=h_nar[:, t % 2, :], in1=n_s, op=ALU.subtract)
            zd = sb.tile([BL, H], F32, tag="zd")
            nc.vector.tensor_tensor(out=zd, in0=z_s, in1=d_t, op=ALU.mult)
            nc.vector.tensor_tensor(out=h_nar[:, (t + 1) % 2, :], in0=n_s, in1=zd, op=ALU.add)
            for kt in range(KT):
                hps = pt_pool.tile([128, BL], F32, tag="tps")
                nc.tensor.transpose(hps, h_nar[:, (t + 1) % 2, kt * 128:(kt + 1) * 128],
                                    ident[0:BL, 0:BL])
                nc.vector.tensor_copy(hT_hist[:, kt, t + 1, :], hps)

        def emit_logits_mtile(mt):
            # hT16 for this m-tile
            for kt in range(KT):
                nc.vector.tensor_copy(
                    hT16[:, kt, :],
                    hT_hist[:, kt, 1 + mt * 32:1 + (mt + 1) * 32, :].rearrange("p t b -> p (t b)"))
            for vc, (v0, vn) in enumerate(VCHUNKS):
                ps = pl_pool.tile([128, NV], F32, tag="pl")
                for kt in range(KT):
                    w16 = ph2sb.tile([128, NV], F16, tag="w16")
                    nc.sync.dma_start(out=w16[:, 0:vn],
                                      in_=d_outwT[kt * 128:(kt + 1) * 128, v0:v0 + vn])
                    nc.tensor.matmul(ps[:, 0:vn], hT16[:, kt, :], w16[:, 0:vn],
                                     start=(kt == 0), stop=False)
                obr = ph2sb.tile([1, NV], F16, tag="obr")
                nc.sync.dma_start(out=obr[:, 0:vn], in_=d_ob[vc:vc + 1, 0:vn])
                nc.tensor.matmul(ps[:, 0:vn], ones16, obr[:, 0:vn],
                                 start=False, stop=True)
                scr = scrsb.tile([128, NV], F32, tag="scr")
                nc.scalar.activation(scr[:, 0:vn], ps[:, 0:vn], AF.Exp,
                                     bias=neg25, scale=1.0,
                                     accum_out=zp[:, mt, vc:vc + 1])
                nc.vector.tensor_copy(logits16[:, v0:v0 + vn], ps[:, 0:vn])

        def emit_logsoftmax_mtile(mt):
            zs = sb.tile([128, 1], F32, tag="zsum")
            nc.vector.tensor_reduce(zs, zp[:, mt, :], axis=AX.X, op=ALU.add)
            lse = sb.tile([128, 1], F32, tag="lse")
            nc.scalar.activation(lse, zs, AF.Ln)
            nlse = sb.tile([128, 1], F32, tag="nlse")
            nc.vector.tensor_scalar(out=nlse, in0=lse, scalar1=LOGIT_SHIFT,
                                    scalar2=-1.0, op0=ALU.add, op1=ALU.mult)
            for vc, (v0, vn) in enumerate(VCHUNKS):
                ob = outsb.tile([128, NV], F32, tag="ob")
                nc.scalar.activation(ob[:, 0:vn], logits16[:, v0:v0 + vn],
                                     AF.Identity, bias=nlse, scale=1.0)
                nc.sync.dma_start(out=d_out[mt * 128:(mt + 1) * 128, v0:v0 + vn],
                                  in_=ob[:, 0:vn])



        for t in range(32):
            emit_step(t)
        emit_logits_mtile(0)
        for t in range(32, 64):
            emit_step(t)
        emit_logsoftmax_mtile(0)
        emit_logits_mtile(1)
        emit_logsoftmax_mtile(1)

        # final hidden state out: hT_hist[:, :, 64, :] -> [b, h]
        hfin = sb.tile([BL, H], F32, tag="hfin")
        nc.vector.tensor_copy(hfin, h_nar[:, 0, :])
        nc.sync.dma_start(out=d_hT.ap(), in_=hfin)

    nc.compile()
    return nc


def _host_inputs(core, encoder_outputs, encoder_hidden, target_tensor,
                 emb, Wa_w, Wa_b, Ua_w, Ua_b, Va_w, Va_b,
                 W_ih, W_hh, b_ih, b_hh, out_w, out_b):
    bsl = slice(core * BL, (core + 1) * BL)
    keys = np.ascontiguousarray(encoder_outputs[bsl])
    h0 = np.ascontiguousarray(encoder_hidden[0, bsl])
    tok = np.concatenate([np.zeros((BL, 1), np.int64),
                          np.asarray(target_tensor[bsl, :T - 1], np.int64)], axis=1)
    eg = emb[tok.T.reshape(-1)]                      # [T*BL, H], m = t*BL+b
    wmovT = np.concatenate([Wa_w.T, W_hh.T], axis=1)  # [H, 4H]
    wihaT = np.ascontiguousarray(W_ih[:, :H].T)
    wihcT = np.ascontiguousarray(W_ih[:, H:].T)
    uaT = np.ascontiguousarray(Ua_w.T)
    bihh = np.stack([b_ih[:H] + b_hh[:H], b_ih[H:2 * H] + b_hh[H:2 * H],
                     b_ih[2 * H:]])
    bhhn = b_hh[2 * H:][None]
    uabT = np.ascontiguousarray((Ua_b + Wa_b).reshape(KT, 128).T)
    vaT = np.ascontiguousarray(Va_w.reshape(KT, 128).T)
    outwT = np.ascontiguousarray(out_w.T.astype(np.float16))
    ob = np.zeros((63, 512), np.float16)
    ob.reshape(-1)[:V] = out_b.astype(np.float16)
    f = np.float32
    return {
        "keys": keys.astype(f), "h0": h0.astype(f), "eg": np.asarray(eg, f),
        "wmovT": np.ascontiguousarray(wmovT, f), "wihaT": wihaT.astype(f),
        "wihcT": wihcT.astype(f), "uaT": uaT.astype(f), "bihh": bihh.astype(f),
        "bhhn": bhhn.astype(f), "uabT": uabT.astype(f), "vaT": vaT.astype(f),
        "outwT": outwT, "ob": ob,
    }


_CACHE = {}


def kernel(encoder_outputs, encoder_hidden, target_tensor, max_len,
           emb, Wa_w, Wa_b, Ua_w, Ua_b, Va_w, Va_b,
           W_ih, W_hh, b_ih, b_hh, out_w, out_b, _trace=False):
    assert int(max_len) == T
    args = [np.asarray(x) for x in
            (encoder_outputs, encoder_hidden, target_tensor, emb, Wa_w, Wa_b,
             Ua_w, Ua_b, Va_w, Va_b, W_ih, W_hh, b_ih, b_hh, out_w, out_b)]
    if "nc" not in _CACHE:
        _CACHE["nc"] = _build_program()
    nc = _CACHE["nc"]
    in_maps = [_host_inputs(c, *args) for c in range(NCORES)]
    r = run_bass_kernel_spmd(nc, in_maps, list(range(NCORES)), trace=_trace)
    dec = np.empty((B, T, V), np.float32)
    attn = np.empty((B, T, S), np.float32)
    hout = np.empty((1, B, H), np.float32)
    for c in range(NCORES):
        o = r.results[c]
        bsl = slice(c * BL, (c + 1) * BL)
        dec[bsl] = o["out"].reshape(T, BL, V).transpose(1, 0, 2)
        attn[bsl] = o["attn"]
        hout[0, bsl] = o["hout"]
    kernel.last_results = r
    return dec, hout, attn


# revision 26
# speedup vs baseline: 50.6499x; 50.6499x over previous
"""AttnDecoderRNN Trainium2 kernel.

B=32, S=64, T=64, H=512, V=32000. 8 NeuronCores, batch-sharded (4 per core).

Per core:
  Phase 0: precompute Ua@keys (transposed), gie = E@W_ihA.T + bias,
           KW = keys@W_ihC.T (pair-stacked), load combined [Wa|W_hh].T.
  Phase 1: 64 sequential GRU+attention steps. Gate matmuls keep h as the
           (tiny) stationary operand and stream the weights; the four gate
           regions go to four PSUM partition bands via tile_position
           col-groups so they run concurrently on the PE sub-arrays.
  Phase 2: logits = hT @ out_w.T in fp16 (batched over 32 timesteps per
           m-tile), log_softmax with a constant 25.0 shift (exact: shift
           cancels), fused exp+accumulate on ACT.
"""

import math
import os
import sys
from contextlib import ExitStack

import numpy as np

sys.path.insert(0, "/opt/trn_rl_repo")

import concourse.bass as bass
import concourse.bacc as bacc
import concourse.mybir as mybir
import concourse.tile as tile
from concourse.bass_utils import run_bass_kernel_spmd
from concourse.masks import make_identity

F32 = mybir.dt.float32
F16 = mybir.dt.float16
AF = mybir.ActivationFunctionType
ALU = mybir.AluOpType
AX = mybir.AxisListType

B, S, T, H, V = 32, 64, 64, 512, 32000
NCORES = 8
BL = B // NCORES          # 4 batches per core
KT = H // 128             # 4 k-tiles
NV = 512                  # v-chunk
NVC = V // NV             # 62.5 -> handle tail
VCHUNKS = [(i * NV, min(NV, V - i * NV)) for i in range((V + NV - 1) // NV)]
LOGIT_SHIFT = 25.0

USE_COLTILE = True


def _build_program():
    nc = bacc.Bacc("TRN2", target_bir_lowering=False, debug=False,
                   enable_asserts=False, num_devices=NCORES)

    # ---- DRAM I/O ----
    d_keys = nc.dram_tensor("keys", [BL, S, H], F32, kind="ExternalInput")
    d_h0 = nc.dram_tensor("h0", [BL, H], F32, kind="ExternalInput")
    d_eg = nc.dram_tensor("eg", [T * BL, H], F32, kind="ExternalInput")
    d_wmov = nc.dram_tensor("wmovT", [H, 4 * H], F16, kind="ExternalInput")
    d_wihaT = nc.dram_tensor("wihaT", [H, 3 * H], F32, kind="ExternalInput")
    d_wihcT = nc.dram_tensor("wihcT", [H, 3 * H], F32, kind="ExternalInput")
    d_uaT = nc.dram_tensor("uaT", [H, H], F32, kind="ExternalInput")
    d_bihh = nc.dram_tensor("bihh", [3, H], F32, kind="ExternalInput")
    d_bhhn = nc.dram_tensor("bhhn", [1, H], F16, kind="ExternalInput")
    d_uabT = nc.dram_tensor("uabT", [128, KT], F32, kind="ExternalInput")
    d_vaT = nc.dram_tensor("vaT", [128, KT], F16, kind="ExternalInput")
    d_outwT = nc.dram_tensor("outwT", [H, V], F16, kind="ExternalInput")
    d_ob = nc.dram_tensor("ob", [63, 512], F16, kind="ExternalInput")

    d_out = nc.dram_tensor("out", [T * BL, V], F32, kind="ExternalOutput")
    d_attn = nc.dram_tensor("attn", [BL, T, S], F32, kind="ExternalOutput")
    d_hT = nc.dram_tensor("hout", [BL, H], F32, kind="ExternalOutput")

    with tile.TileContext(nc) as tc, ExitStack() as ctx:
        res = ctx.enter_context(tc.tile_pool(name="res", bufs=1))

        # ---------- resident tiles ----------
        ident = res.tile([128, 128], F32)
        make_identity(nc, ident)
        wmov = res.tile([128, KT, 4 * H], F16)       # [Wa|Whh].T  (q r z n)
        for kt in range(KT):
            nc.sync.dma_start(out=wmov[:, kt, :], in_=d_wmov[kt * 128:(kt + 1) * 128, :])
        uabT = res.tile([128, KT], F32)
        nc.sync.dma_start(out=uabT, in_=d_uabT.ap())
        vaT = res.tile([128, KT], F16)
        nc.sync.dma_start(out=vaT, in_=d_vaT.ap())
        bihh = []
        for i in range(3):
            bihh_i = res.tile([1, H], F32, tag=f"bihh{i}", name=f"bihh{i}")
            nc.sync.dma_start(out=bihh_i, in_=d_bihh[i:i + 1, :])
            bihh.append(bihh_i)
        bhhn = res.tile([1, H], F16)
        nc.sync.dma_start(out=bhhn, in_=d_bhhn.ap())
        ones1 = res.tile([1, 128], F32)
        nc.vector.memset(ones1, 1.0)
        ones16 = res.tile([1, 128], F16)
        nc.vector.memset(ones16, 1.0)
        neg25 = res.tile([128, 1], F32)
        nc.vector.memset(neg25, -LOGIT_SHIFT)
        ident16 = res.tile([32, 32], F16)
        make_identity(nc, ident16)

        keysT = res.tile([128, KT, BL * S], F32)     # [h, (b-major, s)]
        uakT = res.tile([128, KT, BL * S], F32)      # Ua@keys + (Ua_b + Wa_b)
        gie = res.tile([128, 2, 3 * H], F16)         # E@W_ihA.T + bihh, rows (t%32)*4+b
        kwT = res.tile([128, 2, 3 * H], F16)         # pair-stacked keys@W_ihC.T
        hT_hist = res.tile([128, KT, T + 1, BL], F16)
        h_nar = res.tile([BL, 2, H], F32)
        wT4 = res.tile([128, 2, 4], F16)
        nc.vector.memset(wT4, 0.0)

        # ---------- phase 0 ----------
        with tc.tile_pool(name="p0sb", bufs=3) as p0sb, \
             tc.tile_pool(name="p0ps", bufs=1, space="PSUM") as p0ps, \
             tc.tile_pool(name="p0w", bufs=3) as p0w:

            # keys -> sbuf natural [s, b, h]
            keys_nat = p0sb.tile([S, BL, H], F32, tag="knat")
            nc.sync.dma_start(out=keys_nat, in_=d_keys.ap().rearrange("b s h -> s b h"))
            # h0
            nc.sync.dma_start(out=h_nar[:, 0, :], in_=d_h0.ap())
            # E gathered [m, h], m = t*BL + b ; two m-tiles
            e_nat = p0sb.tile([128, 2, H], F32, tag="enat")
            for mt in range(2):
                nc.sync.dma_start(out=e_nat[:, mt, :], in_=d_eg[mt * 128:(mt + 1) * 128, :])

            # transposes: keysT
            for b in range(BL):
                for kt in range(KT):
                    ps = p0ps.tile([128, 128], F32, tag="tp")
                    nc.tensor.transpose(ps[:, 0:S], keys_nat[:, b, kt * 128:(kt + 1) * 128],
                                        ident[0:S, 0:S])
                    nc.vector.tensor_copy(keysT[:, kt, b * S:(b + 1) * S], ps[:, 0:S])
            # transposes: E_T [k, m]
            eT = p0sb.tile([128, KT, 2 * 128], F32, tag="eT")
            for mt in range(2):
                for kt in range(KT):
                    ps = p0ps.tile([128, 128], F32, tag="tp")
                    nc.tensor.transpose(ps, e_nat[:, mt, kt * 128:(kt + 1) * 128],
                                        ident)
                    nc.vector.tensor_copy(eT[:, kt, mt * 128:(mt + 1) * 128], ps)
            # h0 transposed into hist slot 0
            for kt in range(KT):
                ps = p0ps.tile([128, 128], F32, tag="tp")
                nc.tensor.transpose(ps[:, 0:BL], h_nar[:, 0, kt * 128:(kt + 1) * 128],
                                    ident[0:BL, 0:BL])
                nc.vector.tensor_copy(hT_hist[:, kt, 0, :], ps[:, 0:BL])

            # uakT = Ua @ keys.T + (Ua_b + Wa_b) per-partition
            for it in range(KT):
                ps = p0ps.tile([128, BL * S], F32, tag="uak")
                for kt in range(KT):
                    w = p0w.tile([128, 128], F32, tag="uaw")
                    nc.sync.dma_start(out=w, in_=d_uaT[kt * 128:(kt + 1) * 128,
                                                       it * 128:(it + 1) * 128])
                    nc.tensor.matmul(ps, w, keysT[:, kt, :],
                                     start=(kt == 0), stop=(kt == KT - 1))
                nc.vector.tensor_scalar_add(uakT[:, it, :], ps, uabT[:, it:it + 1])

            # gie = E @ W_ihA.T + bihh   [m, 3H]
            for mt in range(2):
                for ch in range(3):
                    ps = p0ps.tile([128, 512], F32, tag="gie")
                    for kt in range(KT):
                        w = p0w.tile([128, 512], F32, tag="wstream")
                        nc.sync.dma_start(out=w, in_=d_wihaT[kt * 128:(kt + 1) * 128,
                                                             ch * 512:(ch + 1) * 512])
                        nc.tensor.matmul(ps, eT[:, kt, mt * 128:(mt + 1) * 128], w,
                                         start=(kt == 0), stop=False)
                    nc.tensor.matmul(ps, ones1, bihh[ch],
                                     start=False, stop=True)
                    nc.vector.tensor_copy(gie[:, mt, ch * 512:(ch + 1) * 512], ps)

            # kwT = keys @ W_ihC.T  pair-stacked [(2b,s), 3H]
            for pr in range(2):
                for ch in range(3):
                    ps = p0ps.tile([128, 512], F32, tag="kw")
                    for kt in range(KT):
                        w = p0w.tile([128, 512], F32, tag="wstream")
                        nc.sync.dma_start(out=w, in_=d_wihcT[kt * 128:(kt + 1) * 128,
                                                             ch * 512:(ch + 1) * 512])
                        nc.tensor.matmul(ps, keysT[:, kt, pr * 128:(pr + 1) * 128], w,
                                         start=(kt == 0), stop=(kt == KT - 1))
                    nc.vector.tensor_copy(kwT[:, pr, ch * 512:(ch + 1) * 512], ps)

        # ---------- phases 1+2 ----------
        # band layout in pg psum tile [128, 1024]:
        #  rows 0:4   free 0:512 q     | free 512:1024 nacc (KW_n + gie_n)
        #  rows 32:36 free 0:512 r     (Wmov_r + KW_r + gie_r)
        #  rows 64:68 free 0:512 z
        #  rows 96:100 free 0:512 ghn  (Wmov_n + b_hh_n)
        BANDQ, BANDR, BANDZ, BANDN = 0, 32, 64, 96

        def tp(g):
            return (0, g) if USE_COLTILE else (0, 0)

        pg_pool = ctx.enter_context(tc.tile_pool(name="pg", bufs=1, space="PSUM"))
        pt_pool = ctx.enter_context(tc.tile_pool(name="pt", bufs=1, space="PSUM"))
        sc_pool = ctx.enter_context(tc.tile_pool(name="sc", bufs=1, space="PSUM"))
        pl_pool = ctx.enter_context(tc.tile_pool(name="pl", bufs=3, space="PSUM"))
        sb = ctx.enter_context(tc.tile_pool(name="stepsb", bufs=1))
        sb2 = ctx.enter_context(tc.tile_pool(name="stepsb2", bufs=2))
        l16_pool = ctx.enter_context(tc.tile_pool(name="l16", bufs=1))
        ph2sb = ctx.enter_context(tc.tile_pool(name="ph2sb", bufs=10))
        scrsb = ctx.enter_context(tc.tile_pool(name="scrsb", bufs=2))
        outsb = ctx.enter_context(tc.tile_pool(name="outsb", bufs=2))

        logits16 = l16_pool.tile([128, V], F16)
        zp = l16_pool.tile([128, 2, len(VCHUNKS)], F32)

        def emit_step(t):
            pg = pg_pool.tile([128, 512], F32, tag="pg")
            hT_prev = hT_hist[:, :, t, :]
            # qT computed directly: psum[it][h%128, b] = sum_k Wa.T[k, it*128+i] h[b, k]
            qt_ps = pt_pool.tile([128, KT, BL], F32, tag="qtps")
            for it in range(KT):
                for kt in range(KT):
                    nc.tensor.matmul(qt_ps[:, it, :], wmov[:, kt, it * 128:(it + 1) * 128],
                                     hT_prev[:, kt, :],
                                     start=(kt == 0), stop=(kt == KT - 1),
                                     tile_position=tp(0))
            # r band (Wmov part; KW/gie later)
            for kt in range(KT):
                nc.tensor.matmul(pg[BANDR:BANDR + BL, 0:512], hT_prev[:, kt, :],
                                 wmov[:, kt, 512:1024],
                                 start=(kt == 0), stop=False, tile_position=tp(32))
            # z band
            for kt in range(KT):
                nc.tensor.matmul(pg[BANDZ:BANDZ + BL, 0:512], hT_prev[:, kt, :],
                                 wmov[:, kt, 1024:1536],
                                 start=(kt == 0), stop=False, tile_position=tp(64))
            # ghn band + b_hh_n
            for kt in range(KT):
                nc.tensor.matmul(pg[BANDN:BANDN + BL, 0:512], hT_prev[:, kt, :],
                                 wmov[:, kt, 1536:2048],
                                 start=(kt == 0), stop=False, tile_position=tp(96))
            nc.tensor.matmul(pg[BANDN:BANDN + BL, 0:512], ones16[:, 0:BL], bhhn,
                             start=False, stop=True, tile_position=tp(96))

            # ---- attention ----
            s_tanh = sb2.tile([128, KT, BL, S], F16, tag="stanh")
            s_pre = sb2.tile([128, KT, BL, S], F32, tag="spre")
            qb = bass.AP(tensor=qt_ps.tensor, offset=qt_ps.offset,
                         ap=[qt_ps.ap[0], [BL, KT], [1, BL], [0, S]])
            nc.vector.tensor_tensor(
                out=s_pre,
                in0=uakT.rearrange("p k (b s) -> p k b s", b=BL),
                in1=qb, op=ALU.add)
            nc.scalar.activation(
                s_tanh.rearrange("p k b s -> p (k b s)"),
                s_pre.rearrange("p k b s -> p (k b s)"), AF.Tanh)
            ps_s = sc_pool.tile([1, BL * S], F32, tag="scores")
            for kt in range(KT):
                nc.tensor.matmul(ps_s, vaT[:, kt:kt + 1],
                                 s_tanh[:, kt, :, :].rearrange("p b s -> p (b s)"),
                                 start=(kt == 0), stop=(kt == KT - 1),
                                 tile_position=tp(0))
            exps = sb2.tile([1, BL, S], F32, tag="exps")
            nc.scalar.activation(exps.rearrange("p b s -> p (b s)"), ps_s, AF.Exp)
            zr = sb2.tile([1, BL], F32, tag="zr")
            nc.vector.tensor_reduce(zr, exps, axis=AX.X, op=ALU.add)
            zrec = sb2.tile([1, BL], F32, tag="zrec")
            nc.vector.reciprocal(zrec, zr)
            wn = sb2.tile([1, BL, S], F32, tag="wn")
            zb = bass.AP(tensor=zrec.tensor, offset=zrec.offset,
                         ap=[zrec.ap[0], [1, BL], [0, S]])
            nc.vector.tensor_tensor(out=wn, in0=exps, in1=zb, op=ALU.mult)
            nc.sync.dma_start(out=d_attn[:, t, :], in_=wn)
            # w transposed into the zero-padded stationary tiles
            w_ps = pt_pool.tile([128, 2], F32, tag="wps")
            for pr in range(2):
                nc.tensor.transpose(w_ps[:, pr:pr + 1],
                                    wn.rearrange("p b s -> p (b s)")[:, pr * 128:(pr + 1) * 128],
                                    ident[0:1, 0:1])
            pstride = wT4.ap[0][0]
            wlo = bass.AP(tensor=wT4.tensor, offset=wT4[0:S, 0:1, 0:1].offset,
                          ap=[[pstride, S], [6, 2], [1, 1]])
            nc.vector.tensor_copy(wlo, w_ps[0:S, 0:2])
            whi = bass.AP(tensor=wT4.tensor, offset=wT4[S:128, 0:1, 1:2].offset,
                          ap=[[pstride, 128 - S], [6, 2], [1, 1]])
            nc.vector.tensor_copy(whi, w_ps[S:128, 0:2])

            # ---- KW + gie accumulation into bands ----
            t32 = t % 32
            mt = t // 32
            gie_st = sb2.tile([BL, 3 * H], F16, tag="giest")
            nc.sync.dma_start(out=gie_st, in_=gie[4 * t32:4 * t32 + BL, mt, :])
            for ch, band, grp in ((0, BANDR, 32), (1, BANDZ, 64)):
                for pr in range(2):
                    nc.tensor.matmul(pg[band:band + BL, 0:512], wT4[:, pr, :],
                                     kwT[:, pr, ch * 512:(ch + 1) * 512],
                                     start=False, stop=False, tile_position=tp(grp))
                nc.tensor.matmul(pg[band:band + BL, 0:512], ident16[0:BL, 0:BL],
                                 gie_st[:, ch * 512:(ch + 1) * 512],
                                 start=False, stop=True, tile_position=tp(grp))
            # nacc band (free 512:1024)
            for pr in range(2):
                nc.tensor.matmul(pg[BANDQ:BANDQ + BL, 0:512], wT4[:, pr, :],
                                 kwT[:, pr, 1024:1536],
                                 start=(pr == 0), stop=False, tile_position=tp(0))
            nc.tensor.matmul(pg[BANDQ:BANDQ + BL, 0:512], ident16[0:BL, 0:BL],
                             gie_st[:, 1024:1536],
                             start=False, stop=True, tile_position=tp(0))

            # ---- gates ----
            # rz_t = tanh(x/2) over both bands in one op; sigma(x) = .5*rz_t+.5
            # folded into the stt ops below.
            tr_s = sb.tile([BL, H], F32, tag="rs")
            nc.scalar.activation(tr_s, pg[BANDR:BANDR + BL, 0:512], AF.Tanh, scale=0.5)
            tz_s = sb.tile([BL, H], F32, tag="zs")
            nc.scalar.activation(tz_s, pg[BANDZ:BANDZ + BL, 0:512], AF.Tanh, scale=0.5)
            tr = tr_s
            tz = tz_s
            # nh2 = (tanh_r + 1) * ghn   (= 2*r*ghn)
            nh = sb.tile([BL, H], F32, tag="nh")
            nc.vector.scalar_tensor_tensor(out=nh, in0=tr, scalar=1.0, op0=ALU.add,
                                           in1=pg[BANDN:BANDN + BL, 0:512], op1=ALU.mult)
            # npre = nh2*0.5 + nacc
            npre = sb.tile([BL, H], F32, tag="npre")
            nc.vector.scalar_tensor_tensor(out=npre, in0=nh, scalar=0.5, op0=ALU.mult,
                                           in1=pg[BANDQ:BANDQ + BL, 0:512], op1=ALU.add)
            n_s = sb.tile([BL, H], F32, tag="ns")
            nc.scalar.activation(n_s, npre, AF.Tanh)
            # h' = .5*(h+n) + .5*tz*(h-n)
            d_t = sb.tile([BL, H], F32, tag="dt")
            nc.vector.tensor_tensor(out=d_t, in0=h_nar[:, t % 2, :], in1=n_s, op=ALU.subtract)
            a_t = sb.tile([BL, H], F32, tag="at")
            nc.vector.tensor_tensor(out=a_t, in0=h_nar[:, t % 2, :], in1=n_s, op=ALU.add)
            zd = sb.tile([BL, H], F32, tag="zd")
            nc.vector.scalar_tensor_tensor(out=zd, in0=tz, scalar=0.5, op0=ALU.mult,
                                           in1=d_t, op1=ALU.mult)
            nc.vector.scalar_tensor_tensor(out=h_nar[:, (t + 1) % 2, :], in0=a_t,
                                           scalar=0.5, op0=ALU.mult, in1=zd, op1=ALU.add)
            nc.tensor.matmul(qt_ps[0:BL, 0, :], ident[0:BL, 0:BL], npre[:, 0:BL],
                             start=True, stop=True, tile_position=tp(0))
            nc.tensor.matmul(qt_ps[0:BL, 1, :], ident[0:BL, 0:BL], d_t[:, 0:BL],
                             start=True, stop=True, tile_position=tp(0))
            h_ps = pt_pool.tile([128, KT, BL], F32, tag="hps")
            for kt in range(KT):
                nc.tensor.transpose(h_ps[:, kt, :], h_nar[:, (t + 1) % 2, kt * 128:(kt + 1) * 128],
                                    ident[0:BL, 0:BL])
            nc.vector.tensor_copy(hT_hist[:, :, t + 1, :], h_ps)

        def emit_logits_mtile(mt):
            for vc, (v0, vn) in enumerate(VCHUNKS):
                ps = pl_pool.tile([128, NV], F32, tag="pl")
                for kt in range(KT):
                    w16 = ph2sb.tile([128, NV], F16, tag="w16")
                    nc.sync.dma_start(out=w16[:, 0:vn],
                                      in_=d_outwT[kt * 128:(kt + 1) * 128, v0:v0 + vn])
                    nc.tensor.matmul(
                        ps[:, 0:vn],
                        hT_hist[:, kt, 1 + mt * 32:1 + (mt + 1) * 32, :].rearrange("p t b -> p (t b)"),
                        w16[:, 0:vn], start=(kt == 0), stop=False)
                obr = ph2sb.tile([1, NV], F16, tag="obr")
                nc.sync.dma_start(out=obr[:, 0:vn], in_=d_ob[vc:vc + 1, 0:vn])
                nc.tensor.matmul(ps[:, 0:vn], ones16, obr[:, 0:vn],
                                 start=False, stop=True)
                scr = scrsb.tile([128, NV], F32, tag="scr")
                nc.scalar.activation(scr[:, 0:vn], ps[:, 0:vn], AF.Exp,
                                     bias=neg25, scale=1.0,
                                     accum_out=zp[:, mt, vc:vc + 1])
                nc.vector.tensor_copy(logits16[:, v0:v0 + vn], ps[:, 0:vn])

        def emit_logsoftmax_mtile(mt):
            zs = sb.tile([128, 1], F32, tag="zsum")
            nc.vector.tensor_reduce(zs, zp[:, mt, :], axis=AX.X, op=ALU.add)
            lse = sb.tile([128, 1], F32, tag="lse")
            nc.scalar.activation(lse, zs, AF.Ln)
            nlse = sb.tile([128, 1], F32, tag="nlse")
            nc.vector.tensor_scalar(out=nlse, in0=lse, scalar1=LOGIT_SHIFT,
                                    scalar2=-1.0, op0=ALU.add, op1=ALU.mult)
            for vc, (v0, vn) in enumerate(VCHUNKS):
                ob = outsb.tile([128, NV], F32, tag="ob")
                nc.vector.tensor_scalar_add(ob[:, 0:vn], logits16[:, v0:v0 + vn], nlse)
                nc.sync.dma_start(out=d_out[mt * 128:(mt + 1) * 128, v0:v0 + vn],
                                  in_=ob[:, 0:vn])



        for t in range(32):
            emit_step(t)
        emit_logits_mtile(0)
        for t in range(32, 64):
            emit_step(t)
        emit_logsoftmax_mtile(0)
        emit_logits_mtile(1)
        emit_logsoftmax_mtile(1)

        # final hidden state out: hT_hist[:, :, 64, :] -> [b, h]
        hfin = sb.tile([BL, H], F32, tag="hfin")
        nc.vector.tensor_copy(hfin, h_nar[:, 0, :])
        nc.sync.dma_start(out=d_hT.ap(), in_=hfin)

    nc.compile()
    return nc


def _host_inputs(core, encoder_outputs, encoder_hidden, target_tensor,
                 emb, Wa_w, Wa_b, Ua_w, Ua_b, Va_w, Va_b,
                 W_ih, W_hh, b_ih, b_hh, out_w, out_b):
    bsl = slice(core * BL, (core + 1) * BL)
    keys = np.ascontiguousarray(encoder_outputs[bsl])
    h0 = np.ascontiguousarray(encoder_hidden[0, bsl])
    tok = np.concatenate([np.zeros((BL, 1), np.int64),
                          np.asarray(target_tensor[bsl, :T - 1], np.int64)], axis=1)
    eg = emb[tok.T.reshape(-1)]                      # [T*BL, H], m = t*BL+b
    wmovT = np.concatenate([Wa_w.T, W_hh.T], axis=1)  # [H, 4H]
    wihaT = np.ascontiguousarray(W_ih[:, :H].T)
    wihcT = np.ascontiguousarray(W_ih[:, H:].T)
    uaT = np.ascontiguousarray(Ua_w.T)
    bihh = np.stack([b_ih[:H] + b_hh[:H], b_ih[H:2 * H] + b_hh[H:2 * H],
                     b_ih[2 * H:]])
    bhhn = b_hh[2 * H:][None]
    uabT = np.ascontiguousarray((Ua_b + Wa_b).reshape(KT, 128).T)
    vaT = np.ascontiguousarray(Va_w.reshape(KT, 128).T)
    outwT = np.ascontiguousarray(out_w.T.astype(np.float16))
    ob = np.zeros((63, 512), np.float16)
    ob.reshape(-1)[:V] = out_b.astype(np.float16)
    f = np.float32
    return {
        "keys": keys.astype(f), "h0": h0.astype(f), "eg": np.asarray(eg, f),
        "wmovT": np.ascontiguousarray(wmovT).astype(np.float16), "wihaT": wihaT.astype(f),
        "wihcT": wihcT.astype(f), "uaT": uaT.astype(f), "bihh": bihh.astype(f),
        "bhhn": bhhn.astype(np.float16), "uabT": uabT.astype(f), "vaT": vaT.astype(np.float16),
        "outwT": outwT, "ob": ob,
    }


_CACHE = {}


def kernel(encoder_outputs, encoder_hidden, target_tensor, max_len,
           emb, Wa_w, Wa_b, Ua_w, Ua_b, Va_w, Va_b,
           W_ih, W_hh, b_ih, b_hh, out_w, out_b, _trace=False):
    assert int(max_len) == T
    args = [np.asarray(x) for x in
            (encoder_outputs, encoder_hidden, target_tensor, emb, Wa_w, Wa_b,
             Ua_w, Ua_b, Va_w, Va_b, W_ih, W_hh, b_ih, b_hh, out_w, out_b)]
    if "nc" not in _CACHE:
        _CACHE["nc"] = _build_program()
    nc = _CACHE["nc"]
    in_maps = [_host_inputs(c, *args) for c in range(NCORES)]
    r = run_bass_kernel_spmd(nc, in_maps, list(range(NCORES)), trace=_trace)
    dec = np.empty((B, T, V), np.float32)
    attn = np.empty((B, T, S), np.float32)
    hout = np.empty((1, B, H), np.float32)
    for c in range(NCORES):
        o = r.results[c]
        bsl = slice(c * BL, (c + 1) * BL)
        dec[bsl] = o["out"].reshape(T, BL, V).transpose(1, 0, 2)
        attn[bsl] = o["attn"]
        hout[0, bsl] = o["hout"]
    kernel.last_results = r
    return dec, hout, attn


# revision 27
# speedup vs baseline: 51.8073x; 1.0228x over previous
"""AttnDecoderRNN Trainium2 kernel.

B=32, S=64, T=64, H=512, V=32000. 8 NeuronCores, batch-sharded (4 per core).

Per core:
  Phase 0: precompute Ua@keys (transposed), gie = E@W_ihA.T + bias,
           KW = keys@W_ihC.T (pair-stacked), load combined [Wa|W_hh].T.
  Phase 1: 64 sequential GRU+attention steps. Gate matmuls keep h as the
           (tiny) stationary operand and stream the weights; the four gate
           regions go to four PSUM partition bands via tile_position
           col-groups so they run concurrently on the PE sub-arrays.
  Phase 2: logits = hT @ out_w.T in fp16 (batched over 32 timesteps per
           m-tile), log_softmax with a constant 25.0 shift (exact: shift
           cancels), fused exp+accumulate on ACT.
"""

import math
import os
import sys
from contextlib import ExitStack

import numpy as np

sys.path.insert(0, "/opt/trn_rl_repo")

import concourse.bass as bass
import concourse.bacc as bacc
import concourse.mybir as mybir
import concourse.tile as tile
from concourse.bass_utils import run_bass_kernel_spmd
from concourse.masks import make_identity

F32 = mybir.dt.float32
F16 = mybir.dt.float16
AF = mybir.ActivationFunctionType
ALU = mybir.AluOpType
AX = mybir.AxisListType

B, S, T, H, V = 32, 64, 64, 512, 32000
NCORES = 8
BL = B // NCORES          # 4 batches per core
KT = H // 128             # 4 k-tiles
NV = 512                  # v-chunk
NVC = V // NV             # 62.5 -> handle tail
VCHUNKS = [(i * NV, min(NV, V - i * NV)) for i in range((V + NV - 1) // NV)]
LOGIT_SHIFT = 25.0

USE_COLTILE = True


def _build_program():
    nc = bacc.Bacc("TRN2", target_bir_lowering=False, debug=False,
                   enable_asserts=False, num_devices=NCORES)

    # ---- DRAM I/O ----
    d_keys = nc.dram_tensor("keys", [BL, S, H], F32, kind="ExternalInput")
    d_h0 = nc.dram_tensor("h0", [BL, H], F32, kind="ExternalInput")
    d_eg = nc.dram_tensor("eg", [T * BL, H], F32, kind="ExternalInput")
    d_wmov = nc.dram_tensor("wmovT", [H, 4 * H], F16, kind="ExternalInput")
    d_wihaT = nc.dram_tensor("wihaT", [H, 3 * H], F32, kind="ExternalInput")
    d_wihcT = nc.dram_tensor("wihcT", [H, 3 * H], F32, kind="ExternalInput")
    d_uaT = nc.dram_tensor("uaT", [H, H], F32, kind="ExternalInput")
    d_bihh = nc.dram_tensor("bihh", [3, H], F32, kind="ExternalInput")
    d_bhhn = nc.dram_tensor("bhhn", [1, H], F16, kind="ExternalInput")
    d_uabT = nc.dram_tensor("uabT", [128, KT], F32, kind="ExternalInput")
    d_vaT = nc.dram_tensor("vaT", [128, KT], F16, kind="ExternalInput")
    d_outwT = nc.dram_tensor("outwT", [H, V], F16, kind="ExternalInput")
    d_ob = nc.dram_tensor("ob", [63, 512], F16, kind="ExternalInput")

    d_out = nc.dram_tensor("out", [T * BL, V], F32, kind="ExternalOutput")
    d_attn = nc.dram_tensor("attn", [BL, T, S], F32, kind="ExternalOutput")
    d_hT = nc.dram_tensor("hout", [BL, H], F32, kind="ExternalOutput")

    with tile.TileContext(nc) as tc, ExitStack() as ctx:
        res = ctx.enter_context(tc.tile_pool(name="res", bufs=1))

        # ---------- resident tiles ----------
        ident = res.tile([128, 128], F32)
        make_identity(nc, ident)
        wmov = res.tile([128, KT, 4 * H], F16)       # [Wa|Whh].T  (q r z n)
        for kt in range(KT):
            nc.sync.dma_start(out=wmov[:, kt, :], in_=d_wmov[kt * 128:(kt + 1) * 128, :])
        uabT = res.tile([128, KT], F32)
        nc.sync.dma_start(out=uabT, in_=d_uabT.ap())
        vaT = res.tile([128, KT], F16)
        nc.sync.dma_start(out=vaT, in_=d_vaT.ap())
        bihh = []
        for i in range(3):
            bihh_i = res.tile([1, H], F32, tag=f"bihh{i}", name=f"bihh{i}")
            nc.sync.dma_start(out=bihh_i, in_=d_bihh[i:i + 1, :])
            bihh.append(bihh_i)
        bhhn = res.tile([1, H], F16)
        nc.sync.dma_start(out=bhhn, in_=d_bhhn.ap())
        ones1 = res.tile([1, 128], F32)
        nc.vector.memset(ones1, 1.0)
        ones16 = res.tile([1, 128], F16)
        nc.vector.memset(ones16, 1.0)
        neg25 = res.tile([128, 1], F32)
        nc.vector.memset(neg25, -LOGIT_SHIFT)
        ident16 = res.tile([32, 32], F16)
        make_identity(nc, ident16)

        keysT = res.tile([128, KT, BL * S], F32)     # [h, (b-major, s)]
        uakT = res.tile([128, KT, BL * S], F32)      # Ua@keys + (Ua_b + Wa_b)
        gie = res.tile([128, 2, 3 * H], F16)         # E@W_ihA.T + bihh, rows (t%32)*4+b
        kwT = res.tile([128, 2, 3 * H], F16)         # pair-stacked keys@W_ihC.T
        hT_hist = res.tile([128, KT, T + 1, BL], F16)
        h_nar = res.tile([BL, 2, H], F32)
        wT4 = res.tile([128, 2, 4], F16)
        nc.vector.memset(wT4, 0.0)

        # ---------- phase 0 ----------
        with tc.tile_pool(name="p0sb", bufs=3) as p0sb, \
             tc.tile_pool(name="p0ps", bufs=1, space="PSUM") as p0ps, \
             tc.tile_pool(name="p0w", bufs=3) as p0w:

            # keys -> sbuf natural [s, b, h]
            keys_nat = p0sb.tile([S, BL, H], F32, tag="knat")
            nc.sync.dma_start(out=keys_nat, in_=d_keys.ap().rearrange("b s h -> s b h"))
            # h0
            nc.sync.dma_start(out=h_nar[:, 0, :], in_=d_h0.ap())
            # E gathered [m, h], m = t*BL + b ; two m-tiles
            e_nat = p0sb.tile([128, 2, H], F32, tag="enat")
            for mt in range(2):
                nc.sync.dma_start(out=e_nat[:, mt, :], in_=d_eg[mt * 128:(mt + 1) * 128, :])

            # transposes: keysT
            for b in range(BL):
                for kt in range(KT):
                    ps = p0ps.tile([128, 128], F32, tag="tp")
                    nc.tensor.transpose(ps[:, 0:S], keys_nat[:, b, kt * 128:(kt + 1) * 128],
                                        ident[0:S, 0:S])
                    nc.vector.tensor_copy(keysT[:, kt, b * S:(b + 1) * S], ps[:, 0:S])
            # transposes: E_T [k, m]
            eT = p0sb.tile([128, KT, 2 * 128], F32, tag="eT")
            for mt in range(2):
                for kt in range(KT):
                    ps = p0ps.tile([128, 128], F32, tag="tp")
                    nc.tensor.transpose(ps, e_nat[:, mt, kt * 128:(kt + 1) * 128],
                                        ident)
                    nc.vector.tensor_copy(eT[:, kt, mt * 128:(mt + 1) * 128], ps)
            # h0 transposed into hist slot 0
            for kt in range(KT):
                ps = p0ps.tile([128, 128], F32, tag="tp")
                nc.tensor.transpose(ps[:, 0:BL], h_nar[:, 0, kt * 128:(kt + 1) * 128],
                                    ident[0:BL, 0:BL])
                nc.vector.tensor_copy(hT_hist[:, kt, 0, :], ps[:, 0:BL])

            # uakT = Ua @ keys.T + (Ua_b + Wa_b) per-partition
            for it in range(KT):
                ps = p0ps.tile([128, BL * S], F32, tag="uak")
                for kt in range(KT):
                    w = p0w.tile([128, 128], F32, tag="uaw")
                    nc.sync.dma_start(out=w, in_=d_uaT[kt * 128:(kt + 1) * 128,
                                                       it * 128:(it + 1) * 128])
                    nc.tensor.matmul(ps, w, keysT[:, kt, :],
                                     start=(kt == 0), stop=(kt == KT - 1))
                nc.vector.tensor_scalar_add(uakT[:, it, :], ps, uabT[:, it:it + 1])

            # gie = E @ W_ihA.T + bihh   [m, 3H]
            for mt in range(2):
                for ch in range(3):
                    ps = p0ps.tile([128, 512], F32, tag="gie")
                    for kt in range(KT):
                        w = p0w.tile([128, 512], F32, tag="wstream")
                        nc.sync.dma_start(out=w, in_=d_wihaT[kt * 128:(kt + 1) * 128,
                                                             ch * 512:(ch + 1) * 512])
                        nc.tensor.matmul(ps, eT[:, kt, mt * 128:(mt + 1) * 128], w,
                                         start=(kt == 0), stop=False)
                    nc.tensor.matmul(ps, ones1, bihh[ch],
                                     start=False, stop=True)
                    nc.vector.tensor_copy(gie[:, mt, ch * 512:(ch + 1) * 512], ps)

            # kwT = keys @ W_ihC.T  pair-stacked [(2b,s), 3H]
            for pr in range(2):
                for ch in range(3):
                    ps = p0ps.tile([128, 512], F32, tag="kw")
                    for kt in range(KT):
                        w = p0w.tile([128, 512], F32, tag="wstream")
                        nc.sync.dma_start(out=w, in_=d_wihcT[kt * 128:(kt + 1) * 128,
                                                             ch * 512:(ch + 1) * 512])
                        nc.tensor.matmul(ps, keysT[:, kt, pr * 128:(pr + 1) * 128], w,
                                         start=(kt == 0), stop=(kt == KT - 1))
                    nc.vector.tensor_copy(kwT[:, pr, ch * 512:(ch + 1) * 512], ps)

        # ---------- phases 1+2 ----------
        # band layout in pg psum tile [128, 1024]:
        #  rows 0:4   free 0:512 q     | free 512:1024 nacc (KW_n + gie_n)
        #  rows 32:36 free 0:512 r     (Wmov_r + KW_r + gie_r)
        #  rows 64:68 free 0:512 z
        #  rows 96:100 free 0:512 ghn  (Wmov_n + b_hh_n)
        BANDQ, BANDR, BANDZ, BANDN = 0, 32, 64, 96

        def tp(g):
            return (0, g) if USE_COLTILE else (0, 0)

        pg_pool = ctx.enter_context(tc.tile_pool(name="pg", bufs=1, space="PSUM"))
        pt_pool = ctx.enter_context(tc.tile_pool(name="pt", bufs=1, space="PSUM"))
        sc_pool = ctx.enter_context(tc.tile_pool(name="sc", bufs=1, space="PSUM"))
        pl_pool = ctx.enter_context(tc.tile_pool(name="pl", bufs=3, space="PSUM"))
        sb = ctx.enter_context(tc.tile_pool(name="stepsb", bufs=1))
        sb2 = ctx.enter_context(tc.tile_pool(name="stepsb2", bufs=2))
        l16_pool = ctx.enter_context(tc.tile_pool(name="l16", bufs=1))
        ph2sb = ctx.enter_context(tc.tile_pool(name="ph2sb", bufs=10))
        scrsb = ctx.enter_context(tc.tile_pool(name="scrsb", bufs=2))
        outsb = ctx.enter_context(tc.tile_pool(name="outsb", bufs=2))

        logits16 = l16_pool.tile([128, V], F16)
        zp = l16_pool.tile([128, 2, len(VCHUNKS)], F32)

        def emit_step(t):
            pg = pg_pool.tile([128, 512], F32, tag="pg")
            hT_prev = hT_hist[:, :, t, :]
            # qT computed directly: psum[it][h%128, b] = sum_k Wa.T[k, it*128+i] h[b, k]
            qt_ps = pt_pool.tile([128, KT, BL], F32, tag="qtps")
            for it in range(KT):
                for kt in range(KT):
                    nc.tensor.matmul(qt_ps[:, it, :], wmov[:, kt, it * 128:(it + 1) * 128],
                                     hT_prev[:, kt, :],
                                     start=(kt == 0), stop=(kt == KT - 1),
                                     tile_position=tp(0))
            # r band (Wmov part; KW/gie later)
            for kt in range(KT):
                nc.tensor.matmul(pg[BANDR:BANDR + BL, 0:512], hT_prev[:, kt, :],
                                 wmov[:, kt, 512:1024],
                                 start=(kt == 0), stop=False, tile_position=tp(32))
            # z band
            for kt in range(KT):
                nc.tensor.matmul(pg[BANDZ:BANDZ + BL, 0:512], hT_prev[:, kt, :],
                                 wmov[:, kt, 1024:1536],
                                 start=(kt == 0), stop=False, tile_position=tp(64))
            # ghn band + b_hh_n
            for kt in range(KT):
                nc.tensor.matmul(pg[BANDN:BANDN + BL, 0:512], hT_prev[:, kt, :],
                                 wmov[:, kt, 1536:2048],
                                 start=(kt == 0), stop=False, tile_position=tp(96))
            nc.tensor.matmul(pg[BANDN:BANDN + BL, 0:512], ones16[:, 0:BL], bhhn,
                             start=False, stop=True, tile_position=tp(96))

            # ---- attention ----
            s_tanh = sb2.tile([128, KT, BL, S], F16, tag="stanh")
            s_pre = sb2.tile([128, KT, BL, S], F32, tag="spre")
            qb = bass.AP(tensor=qt_ps.tensor, offset=qt_ps.offset,
                         ap=[qt_ps.ap[0], [BL, KT], [1, BL], [0, S]])
            nc.vector.tensor_tensor(
                out=s_pre,
                in0=uakT.rearrange("p k (b s) -> p k b s", b=BL),
                in1=qb, op=ALU.add)
            nc.scalar.activation(
                s_tanh.rearrange("p k b s -> p (k b s)"),
                s_pre.rearrange("p k b s -> p (k b s)"), AF.Tanh)
            ps_s = sc_pool.tile([1, BL * S], F32, tag="scores")
            for kt in range(KT):
                nc.tensor.matmul(ps_s, vaT[:, kt:kt + 1],
                                 s_tanh[:, kt, :, :].rearrange("p b s -> p (b s)"),
                                 start=(kt == 0), stop=(kt == KT - 1),
                                 tile_position=tp(0))
            exps = sb2.tile([1, BL, S], F32, tag="exps")
            nc.scalar.activation(exps.rearrange("p b s -> p (b s)"), ps_s, AF.Exp)
            zr = sb2.tile([1, BL], F32, tag="zr")
            nc.vector.tensor_reduce(zr, exps, axis=AX.X, op=ALU.add)
            zrec = sb2.tile([1, BL], F32, tag="zrec")
            nc.vector.reciprocal(zrec, zr)
            wn = sb2.tile([1, BL, S], F32, tag="wn")
            zb = bass.AP(tensor=zrec.tensor, offset=zrec.offset,
                         ap=[zrec.ap[0], [1, BL], [0, S]])
            nc.vector.tensor_tensor(out=wn, in0=exps, in1=zb, op=ALU.mult)
            nc.sync.dma_start(out=d_attn[:, t, :], in_=wn)
            # w transposed into the zero-padded stationary tiles
            w_ps = pt_pool.tile([128, 2], F32, tag="wps")
            for pr in range(2):
                nc.tensor.transpose(w_ps[:, pr:pr + 1],
                                    wn.rearrange("p b s -> p (b s)")[:, pr * 128:(pr + 1) * 128],
                                    ident[0:1, 0:1])
            pstride = wT4.ap[0][0]
            wlo = bass.AP(tensor=wT4.tensor, offset=wT4[0:S, 0:1, 0:1].offset,
                          ap=[[pstride, S], [6, 2], [1, 1]])
            nc.vector.tensor_copy(wlo, w_ps[0:S, 0:2])
            whi = bass.AP(tensor=wT4.tensor, offset=wT4[S:128, 0:1, 1:2].offset,
                          ap=[[pstride, 128 - S], [6, 2], [1, 1]])
            nc.vector.tensor_copy(whi, w_ps[S:128, 0:2])

            # ---- KW + gie accumulation into bands ----
            t32 = t % 32
            mt = t // 32
            gie_st = sb2.tile([BL, 3 * H], F16, tag="giest")
            nc.sync.dma_start(out=gie_st, in_=gie[4 * t32:4 * t32 + BL, mt, :])
            # gie adds first (independent of softmax) so the in-order PE
            # runs them during the attention window; KW pairs close each group.
            for ch, band, grp in ((0, BANDR, 32), (1, BANDZ, 64)):
                nc.tensor.matmul(pg[band:band + BL, 0:512], ident16[0:BL, 0:BL],
                                 gie_st[:, ch * 512:(ch + 1) * 512],
                                 start=False, stop=False, tile_position=tp(grp))
            nc.tensor.matmul(pg[BANDQ:BANDQ + BL, 0:512], ident16[0:BL, 0:BL],
                             gie_st[:, 1024:1536],
                             start=True, stop=False, tile_position=tp(0))
            for ch, band, grp in ((0, BANDR, 32), (1, BANDZ, 64)):
                for pr in range(2):
                    nc.tensor.matmul(pg[band:band + BL, 0:512], wT4[:, pr, :],
                                     kwT[:, pr, ch * 512:(ch + 1) * 512],
                                     start=False, stop=(pr == 1), tile_position=tp(grp))
            for pr in range(2):
                nc.tensor.matmul(pg[BANDQ:BANDQ + BL, 0:512], wT4[:, pr, :],
                                 kwT[:, pr, 1024:1536],
                                 start=False, stop=(pr == 1), tile_position=tp(0))

            # ---- gates ----
            # rz_t = tanh(x/2) over both bands in one op; sigma(x) = .5*rz_t+.5
            # folded into the stt ops below.
            tr_s = sb.tile([BL, H], F32, tag="rs")
            nc.scalar.activation(tr_s, pg[BANDR:BANDR + BL, 0:512], AF.Tanh, scale=0.5)
            tz_s = sb.tile([BL, H], F32, tag="zs")
            nc.scalar.activation(tz_s, pg[BANDZ:BANDZ + BL, 0:512], AF.Tanh, scale=0.5)
            tr = tr_s
            tz = tz_s
            # nh2 = (tanh_r + 1) * ghn   (= 2*r*ghn)
            nh = sb.tile([BL, H], F32, tag="nh")
            nc.vector.scalar_tensor_tensor(out=nh, in0=tr, scalar=1.0, op0=ALU.add,
                                           in1=pg[BANDN:BANDN + BL, 0:512], op1=ALU.mult)
            # npre = nh2*0.5 + nacc
            npre = sb.tile([BL, H], F32, tag="npre")
            nc.vector.scalar_tensor_tensor(out=npre, in0=nh, scalar=0.5, op0=ALU.mult,
                                           in1=pg[BANDQ:BANDQ + BL, 0:512], op1=ALU.add)
            n_s = sb.tile([BL, H], F32, tag="ns")
            nc.scalar.activation(n_s, npre, AF.Tanh)
            # h' = .5*(h+n) + .5*tz*(h-n)
            d_t = sb.tile([BL, H], F32, tag="dt")
            nc.vector.tensor_tensor(out=d_t, in0=h_nar[:, t % 2, :], in1=n_s, op=ALU.subtract)
            a_t = sb.tile([BL, H], F32, tag="at")
            nc.vector.tensor_tensor(out=a_t, in0=h_nar[:, t % 2, :], in1=n_s, op=ALU.add)
            zd = sb.tile([BL, H], F32, tag="zd")
            nc.vector.scalar_tensor_tensor(out=zd, in0=tz, scalar=0.5, op0=ALU.mult,
                                           in1=d_t, op1=ALU.mult)
            nc.vector.scalar_tensor_tensor(out=h_nar[:, (t + 1) % 2, :], in0=a_t,
                                           scalar=0.5, op0=ALU.mult, in1=zd, op1=ALU.add)
            nc.tensor.matmul(qt_ps[0:BL, 0, :], ident[0:BL, 0:BL], npre[:, 0:BL],
                             start=True, stop=True, tile_position=tp(0))
            nc.tensor.matmul(qt_ps[0:BL, 1, :], ident[0:BL, 0:BL], d_t[:, 0:BL],
                             start=True, stop=True, tile_position=tp(0))
            h_ps = pt_pool.tile([128, KT, BL], F32, tag="hps")
            for kt in range(KT):
                nc.tensor.transpose(h_ps[:, kt, :], h_nar[:, (t + 1) % 2, kt * 128:(kt + 1) * 128],
                                    ident[0:BL, 0:BL])
            nc.vector.tensor_copy(hT_hist[:, :, t + 1, :], h_ps)

        def emit_logits_mtile(mt):
            for vc, (v0, vn) in enumerate(VCHUNKS):
                ps = pl_pool.tile([128, NV], F32, tag="pl")
                for kt in range(KT):
                    w16 = ph2sb.tile([128, NV], F16, tag="w16")
                    nc.sync.dma_start(out=w16[:, 0:vn],
                                      in_=d_outwT[kt * 128:(kt + 1) * 128, v0:v0 + vn])
                    nc.tensor.matmul(
                        ps[:, 0:vn],
                        hT_hist[:, kt, 1 + mt * 32:1 + (mt + 1) * 32, :].rearrange("p t b -> p (t b)"),
                        w16[:, 0:vn], start=(kt == 0), stop=False)
                obr = ph2sb.tile([1, NV], F16, tag="obr")
                nc.sync.dma_start(out=obr[:, 0:vn], in_=d_ob[vc:vc + 1, 0:vn])
                nc.tensor.matmul(ps[:, 0:vn], ones16, obr[:, 0:vn],
                                 start=False, stop=True)
                scr = scrsb.tile([128, NV], F32, tag="scr")
                nc.scalar.activation(scr[:, 0:vn], ps[:, 0:vn], AF.Exp,
                                     bias=neg25, scale=1.0,
                                     accum_out=zp[:, mt, vc:vc + 1])
                nc.vector.tensor_copy(logits16[:, v0:v0 + vn], ps[:, 0:vn])

        def emit_logsoftmax_mtile(mt):
            zs = sb.tile([128, 1], F32, tag="zsum")
            nc.vector.tensor_reduce(zs, zp[:, mt, :], axis=AX.X, op=ALU.add)
            lse = sb.tile([128, 1], F32, tag="lse")
            nc.scalar.activation(lse, zs, AF.Ln)
            nlse = sb.tile([128, 1], F32, tag="nlse")
            nc.vector.tensor_scalar(out=nlse, in0=lse, scalar1=LOGIT_SHIFT,
                                    scalar2=-1.0, op0=ALU.add, op1=ALU.mult)
            for vc, (v0, vn) in enumerate(VCHUNKS):
                ob = outsb.tile([128, NV], F32, tag="ob")
                nc.vector.tensor_scalar_add(ob[:, 0:vn], logits16[:, v0:v0 + vn], nlse)
                nc.sync.dma_start(out=d_out[mt * 128:(mt + 1) * 128, v0:v0 + vn],
                                  in_=ob[:, 0:vn])



        for t in range(32):
            emit_step(t)
        emit_logits_mtile(0)
        for t in range(32, 64):
            emit_step(t)
        emit_logsoftmax_mtile(0)
        emit_logits_mtile(1)
        emit_logsoftmax_mtile(1)

        # final hidden state out: hT_hist[:, :, 64, :] -> [b, h]
        hfin = sb.tile([BL, H], F32, tag="hfin")
        nc.vector.tensor_copy(hfin, h_nar[:, 0, :])
        nc.sync.dma_start(out=d_hT.ap(), in_=hfin)

    nc.compile()
    return nc


def _host_inputs(core, encoder_outputs, encoder_hidden, target_tensor,
                 emb, Wa_w, Wa_b, Ua_w, Ua_b, Va_w, Va_b,
                 W_ih, W_hh, b_ih, b_hh, out_w, out_b):
    bsl = slice(core * BL, (core + 1) * BL)
    keys = np.ascontiguousarray(encoder_outputs[bsl])
    h0 = np.ascontiguousarray(encoder_hidden[0, bsl])
    tok = np.concatenate([np.zeros((BL, 1), np.int64),
                          np.asarray(target_tensor[bsl, :T - 1], np.int64)], axis=1)
    eg = emb[tok.T.reshape(-1)]                      # [T*BL, H], m = t*BL+b
    wmovT = np.concatenate([Wa_w.T, W_hh.T], axis=1)  # [H, 4H]
    wihaT = np.ascontiguousarray(W_ih[:, :H].T)
    wihcT = np.ascontiguousarray(W_ih[:, H:].T)
    uaT = np.ascontiguousarray(Ua_w.T)
    bihh = np.stack([b_ih[:H] + b_hh[:H], b_ih[H:2 * H] + b_hh[H:2 * H],
                     b_ih[2 * H:]])
    bhhn = b_hh[2 * H:][None]
    uabT = np.ascontiguousarray((Ua_b + Wa_b).reshape(KT, 128).T)
    vaT = np.ascontiguousarray(Va_w.reshape(KT, 128).T)
    outwT = np.ascontiguousarray(out_w.T.astype(np.float16))
    ob = np.zeros((63, 512), np.float16)
    ob.reshape(-1)[:V] = out_b.astype(np.float16)
    f = np.float32
    return {
        "keys": keys.astype(f), "h0": h0.astype(f), "eg": np.asarray(eg, f),
        "wmovT": np.ascontiguousarray(wmovT).astype(np.float16), "wihaT": wihaT.astype(f),
        "wihcT": wihcT.astype(f), "uaT": uaT.astype(f), "bihh": bihh.astype(f),
        "bhhn": bhhn.astype(np.float16), "uabT": uabT.astype(f), "vaT": vaT.astype(np.float16),
        "outwT": outwT, "ob": ob,
    }


_CACHE = {}


def kernel(encoder_outputs, encoder_hidden, target_tensor, max_len,
           emb, Wa_w, Wa_b, Ua_w, Ua_b, Va_w, Va_b,
           W_ih, W_hh, b_ih, b_hh, out_w, out_b, _trace=False):
    assert int(max_len) == T
    args = [np.asarray(x) for x in
            (encoder_outputs, encoder_hidden, target_tensor, emb, Wa_w, Wa_b,
             Ua_w, Ua_b, Va_w, Va_b, W_ih, W_hh, b_ih, b_hh, out_w, out_b)]
    if "nc" not in _CACHE:
        _CACHE["nc"] = _build_program()
    nc = _CACHE["nc"]
    in_maps = [_host_inputs(c, *args) for c in range(NCORES)]
    r = run_bass_kernel_spmd(nc, in_maps, list(range(NCORES)), trace=_trace)
    dec = np.empty((B, T, V), np.float32)
    attn = np.empty((B, T, S), np.float32)
    hout = np.empty((1, B, H), np.float32)
    for c in range(NCORES):
        o = r.results[c]
        bsl = slice(c * BL, (c + 1) * BL)
        dec[bsl] = o["out"].reshape(T, BL, V).transpose(1, 0, 2)
        attn[bsl] = o["attn"]
        hout[0, bsl] = o["hout"]
    kernel.last_results = r
    return dec, hout, attn


# revision 28
# speedup vs baseline: 52.0674x; 1.0050x over previous
"""AttnDecoderRNN Trainium2 kernel.

B=32, S=64, T=64, H=512, V=32000. 8 NeuronCores, batch-sharded (4 per core).

Per core:
  Phase 0: precompute Ua@keys (transposed), gie = E@W_ihA.T + bias,
           KW = keys@W_ihC.T (pair-stacked), load combined [Wa|W_hh].T.
  Phase 1: 64 sequential GRU+attention steps. Gate matmuls keep h as the
           (tiny) stationary operand and stream the weights; the four gate
           regions go to four PSUM partition bands via tile_position
           col-groups so they run concurrently on the PE sub-arrays.
  Phase 2: logits = hT @ out_w.T in fp16 (batched over 32 timesteps per
           m-tile), log_softmax with a constant 25.0 shift (exact: shift
           cancels), fused exp+accumulate on ACT.
"""

import math
import os
import sys
from contextlib import ExitStack

import numpy as np

sys.path.insert(0, "/opt/trn_rl_repo")

import concourse.bass as bass
import concourse.bacc as bacc
import concourse.mybir as mybir
import concourse.tile as tile
from concourse.bass_utils import run_bass_kernel_spmd
from concourse.masks import make_identity

F32 = mybir.dt.float32
F16 = mybir.dt.float16
AF = mybir.ActivationFunctionType
ALU = mybir.AluOpType
AX = mybir.AxisListType

B, S, T, H, V = 32, 64, 64, 512, 32000
NCORES = 8
BL = B // NCORES          # 4 batches per core
KT = H // 128             # 4 k-tiles
NV = 512                  # v-chunk
NVC = V // NV             # 62.5 -> handle tail
VCHUNKS = [(i * NV, min(NV, V - i * NV)) for i in range((V + NV - 1) // NV)]
LOGIT_SHIFT = 25.0

USE_COLTILE = True


def _build_program():
    nc = bacc.Bacc("TRN2", target_bir_lowering=False, debug=False,
                   enable_asserts=False, num_devices=NCORES)

    # ---- DRAM I/O ----
    d_keys = nc.dram_tensor("keys", [BL, S, H], F32, kind="ExternalInput")
    d_h0 = nc.dram_tensor("h0", [BL, H], F32, kind="ExternalInput")
    d_eg = nc.dram_tensor("eg", [T * BL, H], F32, kind="ExternalInput")
    d_wmov = nc.dram_tensor("wmovT", [H, 4 * H], F16, kind="ExternalInput")
    d_wihaT = nc.dram_tensor("wihaT", [H, 3 * H], F32, kind="ExternalInput")
    d_wihcT = nc.dram_tensor("wihcT", [H, 3 * H], F32, kind="ExternalInput")
    d_uaT = nc.dram_tensor("uaT", [H, H], F32, kind="ExternalInput")
    d_bihh = nc.dram_tensor("bihh", [3, H], F32, kind="ExternalInput")
    d_bhhn = nc.dram_tensor("bhhn", [1, H], F16, kind="ExternalInput")
    d_uabT = nc.dram_tensor("uabT", [128, KT], F32, kind="ExternalInput")
    d_vaT = nc.dram_tensor("vaT", [128, KT], F16, kind="ExternalInput")
    d_outwT = nc.dram_tensor("outwT", [H, V], F16, kind="ExternalInput")
    d_ob = nc.dram_tensor("ob", [63, 512], F16, kind="ExternalInput")

    d_out = nc.dram_tensor("out", [T * BL, V], F32, kind="ExternalOutput")
    d_attn = nc.dram_tensor("attn", [BL, T, S], F32, kind="ExternalOutput")
    d_hT = nc.dram_tensor("hout", [BL, H], F32, kind="ExternalOutput")

    with tile.TileContext(nc) as tc, ExitStack() as ctx:
        res = ctx.enter_context(tc.tile_pool(name="res", bufs=1))

        # ---------- resident tiles ----------
        ident = res.tile([128, 128], F32)
        make_identity(nc, ident)
        wmov = res.tile([128, KT, 4 * H], F16)       # [Wa|Whh].T  (q r z n)
        for kt in range(KT):
            nc.sync.dma_start(out=wmov[:, kt, :], in_=d_wmov[kt * 128:(kt + 1) * 128, :])
        uabT = res.tile([128, KT], F32)
        nc.sync.dma_start(out=uabT, in_=d_uabT.ap())
        vaT = res.tile([128, KT], F16)
        nc.sync.dma_start(out=vaT, in_=d_vaT.ap())
        bihh = []
        for i in range(3):
            bihh_i = res.tile([1, H], F32, tag=f"bihh{i}", name=f"bihh{i}")
            nc.sync.dma_start(out=bihh_i, in_=d_bihh[i:i + 1, :])
            bihh.append(bihh_i)
        bhhn = res.tile([1, H], F16)
        nc.sync.dma_start(out=bhhn, in_=d_bhhn.ap())
        ones1 = res.tile([1, 128], F32)
        nc.vector.memset(ones1, 1.0)
        ones16 = res.tile([1, 128], F16)
        nc.vector.memset(ones16, 1.0)
        neg25 = res.tile([128, 1], F32)
        nc.vector.memset(neg25, -LOGIT_SHIFT)
        ident16 = res.tile([32, 32], F16)
        make_identity(nc, ident16)

        keysT = res.tile([128, KT, BL * S], F32)     # [h, (b-major, s)]
        uakT = res.tile([128, KT, BL * S], F32)      # Ua@keys + (Ua_b + Wa_b)
        gie = res.tile([128, 2, 3 * H], F16)         # E@W_ihA.T + bihh, rows (t%32)*4+b
        kwT = res.tile([128, 2, 3 * H], F16)         # pair-stacked keys@W_ihC.T
        hT_hist = res.tile([128, KT, T + 1, BL], F16)
        h_nar = res.tile([BL, 2, H], F32)
        wT4 = res.tile([128, 2, 4], F16)
        nc.vector.memset(wT4, 0.0)

        # ---------- phase 0 ----------
        with tc.tile_pool(name="p0sb", bufs=3) as p0sb, \
             tc.tile_pool(name="p0ps", bufs=1, space="PSUM") as p0ps, \
             tc.tile_pool(name="p0w", bufs=3) as p0w:

            # keys -> sbuf natural [s, b, h]
            keys_nat = p0sb.tile([S, BL, H], F32, tag="knat")
            nc.sync.dma_start(out=keys_nat, in_=d_keys.ap().rearrange("b s h -> s b h"))
            # h0
            nc.sync.dma_start(out=h_nar[:, 0, :], in_=d_h0.ap())
            # E gathered [m, h], m = t*BL + b ; two m-tiles
            e_nat = p0sb.tile([128, 2, H], F32, tag="enat")
            for mt in range(2):
                nc.sync.dma_start(out=e_nat[:, mt, :], in_=d_eg[mt * 128:(mt + 1) * 128, :])

            # transposes: keysT
            for b in range(BL):
                for kt in range(KT):
                    ps = p0ps.tile([128, 128], F32, tag="tp")
                    nc.tensor.transpose(ps[:, 0:S], keys_nat[:, b, kt * 128:(kt + 1) * 128],
                                        ident[0:S, 0:S])
                    nc.vector.tensor_copy(keysT[:, kt, b * S:(b + 1) * S], ps[:, 0:S])
            # transposes: E_T [k, m]
            eT = p0sb.tile([128, KT, 2 * 128], F32, tag="eT")
            for mt in range(2):
                for kt in range(KT):
                    ps = p0ps.tile([128, 128], F32, tag="tp")
                    nc.tensor.transpose(ps, e_nat[:, mt, kt * 128:(kt + 1) * 128],
                                        ident)
                    nc.vector.tensor_copy(eT[:, kt, mt * 128:(mt + 1) * 128], ps)
            # h0 transposed into hist slot 0
            for kt in range(KT):
                ps = p0ps.tile([128, 128], F32, tag="tp")
                nc.tensor.transpose(ps[:, 0:BL], h_nar[:, 0, kt * 128:(kt + 1) * 128],
                                    ident[0:BL, 0:BL])
                nc.vector.tensor_copy(hT_hist[:, kt, 0, :], ps[:, 0:BL])

            # uakT = Ua @ keys.T + (Ua_b + Wa_b) per-partition
            for it in range(KT):
                ps = p0ps.tile([128, BL * S], F32, tag="uak")
                for kt in range(KT):
                    w = p0w.tile([128, 128], F32, tag="uaw")
                    nc.sync.dma_start(out=w, in_=d_uaT[kt * 128:(kt + 1) * 128,
                                                       it * 128:(it + 1) * 128])
                    nc.tensor.matmul(ps, w, keysT[:, kt, :],
                                     start=(kt == 0), stop=(kt == KT - 1))
                nc.vector.tensor_scalar_add(uakT[:, it, :], ps, uabT[:, it:it + 1])

            # gie = E @ W_ihA.T + bihh   [m, 3H]
            for mt in range(2):
                for ch in range(3):
                    ps = p0ps.tile([128, 512], F32, tag="gie")
                    for kt in range(KT):
                        w = p0w.tile([128, 512], F32, tag="wstream")
                        nc.sync.dma_start(out=w, in_=d_wihaT[kt * 128:(kt + 1) * 128,
                                                             ch * 512:(ch + 1) * 512])
                        nc.tensor.matmul(ps, eT[:, kt, mt * 128:(mt + 1) * 128], w,
                                         start=(kt == 0), stop=False)
                    nc.tensor.matmul(ps, ones1, bihh[ch],
                                     start=False, stop=True)
                    nc.vector.tensor_copy(gie[:, mt, ch * 512:(ch + 1) * 512], ps)

            # kwT = keys @ W_ihC.T  pair-stacked [(2b,s), 3H]
            for pr in range(2):
                for ch in range(3):
                    ps = p0ps.tile([128, 512], F32, tag="kw")
                    for kt in range(KT):
                        w = p0w.tile([128, 512], F32, tag="wstream")
                        nc.sync.dma_start(out=w, in_=d_wihcT[kt * 128:(kt + 1) * 128,
                                                             ch * 512:(ch + 1) * 512])
                        nc.tensor.matmul(ps, keysT[:, kt, pr * 128:(pr + 1) * 128], w,
                                         start=(kt == 0), stop=(kt == KT - 1))
                    nc.vector.tensor_copy(kwT[:, pr, ch * 512:(ch + 1) * 512], ps)

        # ---------- phases 1+2 ----------
        # band layout in pg psum tile [128, 1024]:
        #  rows 0:4   free 0:512 q     | free 512:1024 nacc (KW_n + gie_n)
        #  rows 32:36 free 0:512 r     (Wmov_r + KW_r + gie_r)
        #  rows 64:68 free 0:512 z
        #  rows 96:100 free 0:512 ghn  (Wmov_n + b_hh_n)
        BANDQ, BANDR, BANDZ, BANDN = 0, 32, 64, 96

        def tp(g):
            return (0, g) if USE_COLTILE else (0, 0)

        pg_pool = ctx.enter_context(tc.tile_pool(name="pg", bufs=1, space="PSUM"))
        pt_pool = ctx.enter_context(tc.tile_pool(name="pt", bufs=1, space="PSUM"))
        sc_pool = ctx.enter_context(tc.tile_pool(name="sc", bufs=1, space="PSUM"))
        pl_pool = ctx.enter_context(tc.tile_pool(name="pl", bufs=3, space="PSUM"))
        sb = ctx.enter_context(tc.tile_pool(name="stepsb", bufs=1))
        sb2 = ctx.enter_context(tc.tile_pool(name="stepsb2", bufs=2))
        l16_pool = ctx.enter_context(tc.tile_pool(name="l16", bufs=1))
        ph2sb = ctx.enter_context(tc.tile_pool(name="ph2sb", bufs=10))
        scrsb = ctx.enter_context(tc.tile_pool(name="scrsb", bufs=2))
        outsb = ctx.enter_context(tc.tile_pool(name="outsb", bufs=2))

        logits16 = l16_pool.tile([128, V], F16)
        zp = l16_pool.tile([128, 2, len(VCHUNKS)], F32)

        def emit_step(t):
            pg = pg_pool.tile([128, 512], F32, tag="pg")
            hT_prev = hT_hist[:, :, t, :]
            # qT computed directly: psum[it][h%128, b] = sum_k Wa.T[k, it*128+i] h[b, k]
            qt_ps = pt_pool.tile([128, KT, BL], F32, tag="qtps")
            for it in range(KT):
                for kt in range(KT):
                    nc.tensor.matmul(qt_ps[:, it, :], wmov[:, kt, it * 128:(it + 1) * 128],
                                     hT_prev[:, kt, :],
                                     start=(kt == 0), stop=(kt == KT - 1),
                                     tile_position=tp(0))
            # r band (Wmov part; KW/gie later)
            for kt in range(KT):
                nc.tensor.matmul(pg[BANDR:BANDR + BL, 0:512], hT_prev[:, kt, :],
                                 wmov[:, kt, 512:1024],
                                 start=(kt == 0), stop=False, tile_position=tp(32))
            # z band
            for kt in range(KT):
                nc.tensor.matmul(pg[BANDZ:BANDZ + BL, 0:512], hT_prev[:, kt, :],
                                 wmov[:, kt, 1024:1536],
                                 start=(kt == 0), stop=False, tile_position=tp(64))
            # ghn band + b_hh_n
            for kt in range(KT):
                nc.tensor.matmul(pg[BANDN:BANDN + BL, 0:512], hT_prev[:, kt, :],
                                 wmov[:, kt, 1536:2048],
                                 start=(kt == 0), stop=False, tile_position=tp(96))
            nc.tensor.matmul(pg[BANDN:BANDN + BL, 0:512], ones16[:, 0:BL], bhhn,
                             start=False, stop=True, tile_position=tp(96))

            # ---- attention ----
            s_tanh = sb2.tile([128, KT, BL, S], F16, tag="stanh")
            s_pre = sb2.tile([128, KT, BL, S], F32, tag="spre")
            for h2 in range(2):
                qh = qt_ps[:, 2 * h2:2 * h2 + 2, :]
                qb = bass.AP(tensor=qh.tensor, offset=qh.offset,
                             ap=[qh.ap[0], [BL, 2], [1, BL], [0, S]])
                nc.vector.tensor_tensor(
                    out=s_pre[:, 2 * h2:2 * h2 + 2, :, :],
                    in0=uakT[:, 2 * h2:2 * h2 + 2, :].rearrange(
                        "p k (b s) -> p k b s", b=BL),
                    in1=qb, op=ALU.add)
                nc.scalar.activation(
                    s_tanh[:, 2 * h2:2 * h2 + 2, :, :].rearrange("p k b s -> p (k b s)"),
                    s_pre[:, 2 * h2:2 * h2 + 2, :, :].rearrange("p k b s -> p (k b s)"),
                    AF.Tanh)
            ps_s = sc_pool.tile([1, BL * S], F32, tag="scores")
            for kt in range(KT):
                nc.tensor.matmul(ps_s, vaT[:, kt:kt + 1],
                                 s_tanh[:, kt, :, :].rearrange("p b s -> p (b s)"),
                                 start=(kt == 0), stop=(kt == KT - 1),
                                 tile_position=tp(0))
            exps = sb2.tile([1, BL, S], F32, tag="exps")
            nc.scalar.activation(exps.rearrange("p b s -> p (b s)"), ps_s, AF.Exp)
            zr = sb2.tile([1, BL], F32, tag="zr")
            nc.vector.tensor_reduce(zr, exps, axis=AX.X, op=ALU.add)
            zrec = sb2.tile([1, BL], F32, tag="zrec")
            nc.vector.reciprocal(zrec, zr)
            wn = sb2.tile([1, BL, S], F32, tag="wn")
            zb = bass.AP(tensor=zrec.tensor, offset=zrec.offset,
                         ap=[zrec.ap[0], [1, BL], [0, S]])
            nc.vector.tensor_tensor(out=wn, in0=exps, in1=zb, op=ALU.mult)
            nc.sync.dma_start(out=d_attn[:, t, :], in_=wn)
            # w transposed into the zero-padded stationary tiles
            w_ps = pt_pool.tile([128, 2], F32, tag="wps")
            for pr in range(2):
                nc.tensor.transpose(w_ps[:, pr:pr + 1],
                                    wn.rearrange("p b s -> p (b s)")[:, pr * 128:(pr + 1) * 128],
                                    ident[0:1, 0:1])
            pstride = wT4.ap[0][0]
            wlo = bass.AP(tensor=wT4.tensor, offset=wT4[0:S, 0:1, 0:1].offset,
                          ap=[[pstride, S], [6, 2], [1, 1]])
            nc.vector.tensor_copy(wlo, w_ps[0:S, 0:2])
            whi = bass.AP(tensor=wT4.tensor, offset=wT4[S:128, 0:1, 1:2].offset,
                          ap=[[pstride, 128 - S], [6, 2], [1, 1]])
            nc.vector.tensor_copy(whi, w_ps[S:128, 0:2])

            # ---- KW + gie accumulation into bands ----
            t32 = t % 32
            mt = t // 32
            gie_st = sb2.tile([BL, 3 * H], F16, tag="giest")
            nc.sync.dma_start(out=gie_st, in_=gie[4 * t32:4 * t32 + BL, mt, :])
            # gie adds first (independent of softmax) so the in-order PE
            # runs them during the attention window; KW pairs close each group.
            for ch, band, grp in ((0, BANDR, 32), (1, BANDZ, 64)):
                nc.tensor.matmul(pg[band:band + BL, 0:512], ident16[0:BL, 0:BL],
                                 gie_st[:, ch * 512:(ch + 1) * 512],
                                 start=False, stop=False, tile_position=tp(grp))
            nc.tensor.matmul(pg[BANDQ:BANDQ + BL, 0:512], ident16[0:BL, 0:BL],
                             gie_st[:, 1024:1536],
                             start=True, stop=False, tile_position=tp(0))
            for ch, band, grp in ((0, BANDR, 32), (1, BANDZ, 64)):
                for pr in range(2):
                    nc.tensor.matmul(pg[band:band + BL, 0:512], wT4[:, pr, :],
                                     kwT[:, pr, ch * 512:(ch + 1) * 512],
                                     start=False, stop=(pr == 1), tile_position=tp(grp))
            for pr in range(2):
                nc.tensor.matmul(pg[BANDQ:BANDQ + BL, 0:512], wT4[:, pr, :],
                                 kwT[:, pr, 1024:1536],
                                 start=False, stop=(pr == 1), tile_position=tp(0))

            # ---- gates ----
            # rz_t = tanh(x/2) over both bands in one op; sigma(x) = .5*rz_t+.5
            # folded into the stt ops below.
            tr_s = sb.tile([BL, H], F32, tag="rs")
            nc.scalar.activation(tr_s, pg[BANDR:BANDR + BL, 0:512], AF.Tanh, scale=0.5)
            tz_s = sb.tile([BL, H], F32, tag="zs")
            nc.scalar.activation(tz_s, pg[BANDZ:BANDZ + BL, 0:512], AF.Tanh, scale=0.5)
            tr = tr_s
            tz = tz_s
            # nh2 = (tanh_r + 1) * ghn   (= 2*r*ghn)
            nh = sb.tile([BL, H], F32, tag="nh")
            nc.vector.scalar_tensor_tensor(out=nh, in0=tr, scalar=1.0, op0=ALU.add,
                                           in1=pg[BANDN:BANDN + BL, 0:512], op1=ALU.mult)
            # npre = nh2*0.5 + nacc
            npre = sb.tile([BL, H], F32, tag="npre")
            nc.vector.scalar_tensor_tensor(out=npre, in0=nh, scalar=0.5, op0=ALU.mult,
                                           in1=pg[BANDQ:BANDQ + BL, 0:512], op1=ALU.add)
            n_s = sb.tile([BL, H], F32, tag="ns")
            nc.scalar.activation(n_s, npre, AF.Tanh)
            # h' = .5*(h+n) + .5*tz*(h-n)
            d_t = sb.tile([BL, H], F32, tag="dt")
            nc.vector.tensor_tensor(out=d_t, in0=h_nar[:, t % 2, :], in1=n_s, op=ALU.subtract)
            a_t = sb.tile([BL, H], F32, tag="at")
            nc.vector.tensor_tensor(out=a_t, in0=h_nar[:, t % 2, :], in1=n_s, op=ALU.add)
            zd = sb.tile([BL, H], F32, tag="zd")
            nc.vector.scalar_tensor_tensor(out=zd, in0=tz, scalar=0.5, op0=ALU.mult,
                                           in1=d_t, op1=ALU.mult)
            nc.vector.scalar_tensor_tensor(out=h_nar[:, (t + 1) % 2, :], in0=a_t,
                                           scalar=0.5, op0=ALU.mult, in1=zd, op1=ALU.add)
            nc.tensor.matmul(qt_ps[0:BL, 0, :], ident[0:BL, 0:BL], npre[:, 0:BL],
                             start=True, stop=True, tile_position=tp(0))
            nc.tensor.matmul(qt_ps[0:BL, 1, :], ident[0:BL, 0:BL], d_t[:, 0:BL],
                             start=True, stop=True, tile_position=tp(0))
            h_ps = pt_pool.tile([128, KT, BL], F32, tag="hps")
            for kt in range(KT):
                nc.tensor.transpose(h_ps[:, kt, :], h_nar[:, (t + 1) % 2, kt * 128:(kt + 1) * 128],
                                    ident[0:BL, 0:BL])
            nc.vector.tensor_copy(hT_hist[:, :, t + 1, :], h_ps)

        def emit_logits_mtile(mt):
            for vc, (v0, vn) in enumerate(VCHUNKS):
                ps = pl_pool.tile([128, NV], F32, tag="pl")
                for kt in range(KT):
                    w16 = ph2sb.tile([128, NV], F16, tag="w16")
                    nc.sync.dma_start(out=w16[:, 0:vn],
                                      in_=d_outwT[kt * 128:(kt + 1) * 128, v0:v0 + vn])
                    nc.tensor.matmul(
                        ps[:, 0:vn],
                        hT_hist[:, kt, 1 + mt * 32:1 + (mt + 1) * 32, :].rearrange("p t b -> p (t b)"),
                        w16[:, 0:vn], start=(kt == 0), stop=False)
                obr = ph2sb.tile([1, NV], F16, tag="obr")
                nc.sync.dma_start(out=obr[:, 0:vn], in_=d_ob[vc:vc + 1, 0:vn])
                nc.tensor.matmul(ps[:, 0:vn], ones16, obr[:, 0:vn],
                                 start=False, stop=True)
                scr = scrsb.tile([128, NV], F32, tag="scr")
                nc.scalar.activation(scr[:, 0:vn], ps[:, 0:vn], AF.Exp,
                                     bias=neg25, scale=1.0,
                                     accum_out=zp[:, mt, vc:vc + 1])
                nc.vector.tensor_copy(logits16[:, v0:v0 + vn], ps[:, 0:vn])

        def emit_logsoftmax_mtile(mt):
            zs = sb.tile([128, 1], F32, tag="zsum")
            nc.vector.tensor_reduce(zs, zp[:, mt, :], axis=AX.X, op=ALU.add)
            lse = sb.tile([128, 1], F32, tag="lse")
            nc.scalar.activation(lse, zs, AF.Ln)
            nlse = sb.tile([128, 1], F32, tag="nlse")
            nc.vector.tensor_scalar(out=nlse, in0=lse, scalar1=LOGIT_SHIFT,
                                    scalar2=-1.0, op0=ALU.add, op1=ALU.mult)
            for vc, (v0, vn) in enumerate(VCHUNKS):
                ob = outsb.tile([128, NV], F32, tag="ob")
                nc.vector.tensor_scalar_add(ob[:, 0:vn], logits16[:, v0:v0 + vn], nlse)
                nc.sync.dma_start(out=d_out[mt * 128:(mt + 1) * 128, v0:v0 + vn],
                                  in_=ob[:, 0:vn])



        for t in range(32):
            emit_step(t)
        emit_logits_mtile(0)
        for t in range(32, 64):
            emit_step(t)
        emit_logsoftmax_mtile(0)
        emit_logits_mtile(1)
        emit_logsoftmax_mtile(1)

        # final hidden state out: hT_hist[:, :, 64, :] -> [b, h]
        hfin = sb.tile([BL, H], F32, tag="hfin")
        nc.vector.tensor_copy(hfin, h_nar[:, 0, :])
        nc.sync.dma_start(out=d_hT.ap(), in_=hfin)

    nc.compile()
    return nc


def _host_inputs(core, encoder_outputs, encoder_hidden, target_tensor,
                 emb, Wa_w, Wa_b, Ua_w, Ua_b, Va_w, Va_b,
                 W_ih, W_hh, b_ih, b_hh, out_w, out_b):
    bsl = slice(core * BL, (core + 1) * BL)
    keys = np.ascontiguousarray(encoder_outputs[bsl])
    h0 = np.ascontiguousarray(encoder_hidden[0, bsl])
    tok = np.concatenate([np.zeros((BL, 1), np.int64),
                          np.asarray(target_tensor[bsl, :T - 1], np.int64)], axis=1)
    eg = emb[tok.T.reshape(-1)]                      # [T*BL, H], m = t*BL+b
    wmovT = np.concatenate([Wa_w.T, W_hh.T], axis=1)  # [H, 4H]
    wihaT = np.ascontiguousarray(W_ih[:, :H].T)
    wihcT = np.ascontiguousarray(W_ih[:, H:].T)
    uaT = np.ascontiguousarray(Ua_w.T)
    bihh = np.stack([b_ih[:H] + b_hh[:H], b_ih[H:2 * H] + b_hh[H:2 * H],
                     b_ih[2 * H:]])
    bhhn = b_hh[2 * H:][None]
    uabT = np.ascontiguousarray((Ua_b + Wa_b).reshape(KT, 128).T)
    vaT = np.ascontiguousarray(Va_w.reshape(KT, 128).T)
    outwT = np.ascontiguousarray(out_w.T.astype(np.float16))
    ob = np.zeros((63, 512), np.float16)
    ob.reshape(-1)[:V] = out_b.astype(np.float16)
    f = np.float32
    return {
        "keys": keys.astype(f), "h0": h0.astype(f), "eg": np.asarray(eg, f),
        "wmovT": np.ascontiguousarray(wmovT).astype(np.float16), "wihaT": wihaT.astype(f),
        "wihcT": wihcT.astype(f), "uaT": uaT.astype(f), "bihh": bihh.astype(f),
        "bhhn": bhhn.astype(np.float16), "uabT": uabT.astype(f), "vaT": vaT.astype(np.float16),
        "outwT": outwT, "ob": ob,
    }


_CACHE = {}


def kernel(encoder_outputs, encoder_hidden, target_tensor, max_len,
           emb, Wa_w, Wa_b, Ua_w, Ua_b, Va_w, Va_b,
           W_ih, W_hh, b_ih, b_hh, out_w, out_b, _trace=False):
    assert int(max_len) == T
    args = [np.asarray(x) for x in
            (encoder_outputs, encoder_hidden, target_tensor, emb, Wa_w, Wa_b,
             Ua_w, Ua_b, Va_w, Va_b, W_ih, W_hh, b_ih, b_hh, out_w, out_b)]
    if "nc" not in _CACHE:
        _CACHE["nc"] = _build_program()
    nc = _CACHE["nc"]
    in_maps = [_host_inputs(c, *args) for c in range(NCORES)]
    r = run_bass_kernel_spmd(nc, in_maps, list(range(NCORES)), trace=_trace)
    dec = np.empty((B, T, V), np.float32)
    attn = np.empty((B, T, S), np.float32)
    hout = np.empty((1, B, H), np.float32)
    for c in range(NCORES):
        o = r.results[c]
        bsl = slice(c * BL, (c + 1) * BL)
        dec[bsl] = o["out"].reshape(T, BL, V).transpose(1, 0, 2)
        attn[bsl] = o["attn"]
        hout[0, bsl] = o["hout"]
    kernel.last_results = r
    return dec, hout, attn
